# revision 1
# baseline (speedup 1.0000x reference)
"""Trainium2 Bass kernel: pre-LN transformer block (B=4, T=2048, E=1024, H=16, FFN=100).

Sharding (8 NeuronCores): core 2b+g handles batch b, head-group g (8 of 16 heads,
i.e. a 512-wide slice of the QKV output dim / proj input dim).  Both cores of a
pair compute attention + proj partials for all 2048 tokens of their batch; a
per-pair ReduceScatter combines the partials and hands each core half the
tokens, on which it runs LN2 + FFN and writes its [1024, 1024] output shard.

SPMD notes: all 8 cores run one program; per-core behavior differs only via
input data.  The residual is fed as x/2 on both pair members (summed back to x
by the reduce); LN1 uses eps/4 so layernorm(x/2, eps/4) == layernorm(x, eps)
exactly.  b_proj/2 is folded into the proj matmul as an extra K=1 term, and b2
is folded into the FFN second matmul as an extra input row.

Attention layout: scores are computed transposed, S^T[t_k, t_q] = k^T.T @ q^T,
with q^T/k^T in [head_dim, token] layout (from PE-transposed LN output).
Softmax runs without max subtraction (logits are ~N(0, 0.25), safe in fp32):
exp on ScalarE straight out of PSUM with the 1/sqrt(E) scale folded in, causal
masking by multiplying the four diagonal-block patterns, and the denominator
obtained by appending a ones column to V so that the P@V matmul's extra output
row is sum_k P[t_k, t_q].  The reciprocal denominator is broadcast across each
head's 64 partitions with a tiny SBUF->SBUF DMA and applied during the
PSUM->SBUF copy of the attention output.
"""

from contextlib import ExitStack

import numpy as np
import ml_dtypes

import concourse.bass as bass
import concourse.mybir as mybir
import concourse.tile as tile
from concourse.bass_utils import run_bass_kernel_spmd
from concourse.vector_clock import ScopedClock


class SplitDrainTC(tile.TileContext):
    """Works around a walrus codegen limit: an SP CTRL instruction may carry
    only one sync wait, so the kernel-tail drain's waits are split onto
    preceding single-wait nops."""

    def _drain_and_barrier(self, tick_clock, wait_clock):
        probe = self.nc.sync.nop(nofuse=True)
        wait_clock.add_sem_waits(
            probe.ins, ScopedClock({None: tick_clock.global_clock})
        )
        si = probe.ins.sync_info
        waits = list(si.on_wait) if si is not None else []
        if len(waits) > 1:
            si.on_wait = [waits[0]]
            for w in waits[1:]:
                n2 = self.nc.sync.nop(nofuse=True)
                n2.ins.sync_info = mybir.SyncInfo(on_wait=[w], on_update=[])
        self.nc.sync.drain()
        self.nc.all_engine_barrier()
        popped = self.nc._tile_sem_poison_stack.pop()
        assert popped is self._sem_poison
        self.nc.clear_and_free_semaphores(list(self.sems.allocated().values()))
        self.nc.all_engine_barrier()

B, T, E, H, HS, FFN = 4, 2048, 1024, 16, 64, 100
EPS = 1e-5
NCORE = 8
TC = 512            # token chunk
NTC = T // TC       # 4
TS = 128            # token subtile
NSUB = TC // TS     # 4
ET = 128            # embed tile
NET = E // ET       # 8
DSL = E // 2        # per-core qkv output slice (8 heads * 64)
NDT = DSL // 128    # 4 d-tiles (2 heads each)
HPC = H // 2        # 8 heads per core
SCALE = float(E) ** -0.5
NKT = T // TS       # 16 t_k tiles
PAIRS = [[0, 1], [2, 3], [4, 5], [6, 7]]

MM_MODE = "bf16"    # "bf16" | "f32r" | "f32"
DEBUG = False       # add a per-core debug output (pre-reduce proj partial)
AF = mybir.ActivationFunctionType


def _mdt(mode):
    return mybir.dt.bfloat16 if mode == "bf16" else mybir.dt.float32


def _np_mdt(mode):
    return ml_dtypes.bfloat16 if mode == "bf16" else np.float32


def build(mode=MM_MODE):
    f32 = mybir.dt.float32
    mdt = _mdt(mode)

    def mc(ap):
        """Cast an AP for use as a matmul operand."""
        if mode == "f32r":
            return ap.bitcast(mybir.dt.float32r)
        return ap

    nc = bass.Bass(num_devices=NCORE)

    io = {}

    def param(name, shape, dtype):
        io[name] = nc.declare_dram_parameter(name, shape, dtype, isOutput=False)

    param("xr", [T, E], f32)           # x/2
    param("wq", [E, DSL], mdt)
    param("wk", [E, DSL], mdt)
    param("wv", [E, DSL], mdt)
    param("wp", [DSL, E], mdt)
    param("bp", [1, E], mdt)           # b_proj/2
    param("w1", [E, FFN], mdt)
    param("w2e", [FFN + 1, E], mdt)    # w2 with b2 as the extra last row
    param("b1", [FFN, 1], f32)
    param("ln1g", [E, 1], f32)
    param("ln1b", [E, 1], f32)
    param("ln2g", [E, 1], f32)
    param("ln2b", [E, 1], f32)
    param("masks", [TS, NSUB, TC], mdt)
    param("ident", [TS, TS], f32)
    io["out"] = nc.declare_dram_parameter("out", [T // 2, E], f32, isOutput=True)
    if DEBUG:
        io["dbg"] = nc.declare_dram_parameter("dbg", [T, E], f32, isOutput=True)

    with SplitDrainTC(nc) as tc:
        with ExitStack() as ctx:
            _build_tile(ctx, tc, nc, mode, mdt, f32, mc, io)
    _split_waits(nc)
    return nc


def _split_waits(nc, maxw=1):
    """walrus codegen accepts a limited number of sync waits per instruction;
    move the excess onto same-engine NoOps inserted just before."""
    import bass_rust
    n = 0
    for f in nc.m.functions:
        for b in f.blocks:
            new = []
            for inst in b.instructions:
                si = inst.sync_info
                # fixed-length ISA instructions can't carry waits at all
                cap = 0 if isinstance(inst, bass_rust.InstISA) else maxw
                if si is not None and len(si.on_wait) > cap:
                    waits = list(si.on_wait)
                    keep = waits[-cap:] if cap else []
                    excess = waits[:-cap] if cap else waits
                    for w in excess:
                        nop = mybir.InstNoOp(
                            name=f"{inst.name}-wsplit{n}", engine=inst.engine
                        )
                        nop.bass_nofuse = True
                        n += 1
                        nop.sync_info = mybir.SyncInfo(
                            on_wait=[w], on_update=[]
                        )
                        new.append(nop)
                    si.on_wait = keep
                new.append(inst)
            if n:
                b.instructions = new


def _build_tile(ctx, tc, nc, mode, mdt, f32, mc, io):
    xr, out = io["xr"], io["out"]

    def pool(name, bufs, space="SBUF"):
        return ctx.enter_context(tc.tile_pool(name=name, bufs=bufs, space=space))

    # ---- internal DRAM ----
    dram = pool("dram", 1, space="DRAM")
    den_dram_pool = pool("dend", 8, space="DRAM")
    ar_in = dram.tile([T, E], f32, name="ar_in")
    rs_out = [
        dram.tile([TC, E], f32, name="rs0_out"),
        dram.tile([TC, E], f32, name="rs1_out"),
    ]

    # ---- persistent SBUF: weights & constants ----
    wpool = pool("weights", 1)
    wq_sb = wpool.tile([ET, NET, DSL], mdt, name="wq_sb")
    wk_sb = wpool.tile([ET, NET, DSL], mdt, name="wk_sb")
    wv_sb = wpool.tile([ET, NET, DSL], mdt, name="wv_sb")
    nc.sync.dma_start(out=wq_sb, in_=io["wq"].rearrange("(k p) d -> p k d", p=ET))
    nc.sync.dma_start(out=wk_sb, in_=io["wk"].rearrange("(k p) d -> p k d", p=ET))
    nc.sync.dma_start(out=wv_sb, in_=io["wv"].rearrange("(k p) d -> p k d", p=ET))
    wp_sb = wpool.tile([128, NDT, E], mdt, name="wp_sb")
    nc.sync.dma_start(out=wp_sb, in_=io["wp"].rearrange("(k p) d -> p k d", p=128))
    bp_sb = wpool.tile([1, E], mdt, name="bp_sb")
    nc.sync.dma_start(out=bp_sb, in_=io["bp"][:])
    ones1 = wpool.tile([1, TS], mdt, name="ones1")
    nc.vector.memset(ones1, 1.0)
    ones65 = wpool.tile([HS + 1, 1], mdt, name="ones65")
    nc.vector.memset(ones65, 1.0)
    w1_sb = wpool.tile([ET, NET, FFN], mdt, name="w1_sb")
    nc.sync.dma_start(out=w1_sb, in_=io["w1"].rearrange("(k p) d -> p k d", p=ET))
    w2_sb = wpool.tile([FFN + 1, E], mdt, name="w2_sb")
    nc.sync.dma_start(out=w2_sb, in_=io["w2e"][:])
    b1_sb = wpool.tile([FFN, 1], f32, name="b1_sb")
    nc.sync.dma_start(out=b1_sb, in_=io["b1"][:])
    ln_sb = {}
    for nm in ("ln1g", "ln1b", "ln2g", "ln2b"):
        ln_sb[nm] = wpool.tile([ET, NET, 1], f32, name=nm + "_sb")
        nc.sync.dma_start(
            out=ln_sb[nm], in_=io[nm].rearrange("(k p) o -> p k o", p=ET)
        )
    mask_sb = wpool.tile([TS, NSUB, TC], mdt, name="mask_sb")
    nc.sync.dma_start(out=mask_sb, in_=io["masks"][:])
    id_sb = wpool.tile([TS, TS], f32, name="id_sb")
    nc.sync.dma_start(out=id_sb, in_=io["ident"][:])
    eps1_sb = wpool.tile([128, 1], f32, name="eps1_sb")
    nc.vector.memset(eps1_sb, EPS / 4.0)  # LN1 runs on x/2
    eps2_sb = wpool.tile([128, 1], f32, name="eps2_sb")
    nc.vector.memset(eps2_sb, EPS)

    # ---- persistent SBUF: per-chunk K^T, V(+ones), Q^T (separate tiles so
    # attention on chunk c only depends on qkv of chunks <= c) ----
    kv = pool("kv", 1)
    kT_c = [kv.tile([128, NDT, TC], mdt, name=f"kT{c}") for c in range(NTC)]
    vt_c = [kv.tile([128, NSUB, HPC, HS + 1], mdt, name=f"vt{c}")
            for c in range(NTC)]
    qT_c = [kv.tile([128, NDT, TC], mdt, name=f"qT{c}") for c in range(NTC)]

    # ---- working pools ----
    xt_pool = pool("xt", 3)
    mv_pool = pool("mv", 3)
    hT_pool = pool("hT", 2)
    pt_pool = pool("pt", 12)
    attT_pool = pool("attT", 4)
    rb_pool = pool("rb", 3)
    avs_pool = pool("avs", 4)
    rbt_pool = pool("rbt", 2)
    stage_pool = pool("stage", 2)
    f1_pool = pool("f1", 2)
    out_pool = pool("outp", 2)
    ps_mm = pool("ps_mm", 2, space="PSUM")
    ps_tr = pool("ps_tr", 1, space="PSUM")
    ps_sc = pool("ps_sc", 3, space="PSUM")
    ps_av = pool("ps_av", 2, space="PSUM")

    def layer_norm(x_t, eps_tile, out_t=None):
        """(x - mean) * rsqrt(var + eps); in-place unless out_t given."""
        stats = mv_pool.tile([128, 2, nc.vector.BN_STATS_DIM], f32, name="stats")
        xg = x_t.rearrange("p (s q) -> p s q", s=2)
        for s in range(2):
            nc.vector.bn_stats(out=stats[:, s, :], in_=xg[:, s, :])
        mv = mv_pool.tile([128, 2], f32, name="mv")
        nc.vector.bn_aggr(out=mv, in_=stats)
        rstd = mv_pool.tile([128, 1], f32, name="rstd")
        nc.scalar.activation(
            out=rstd, in_=mv[:, 1:2], func=AF.Sqrt, bias=eps_tile, scale=1.0
        )
        nc.vector.reciprocal(out=rstd, in_=rstd)
        if out_t is None:
            out_t = x_t
        nc.vector.tensor_scalar(
            out=out_t, in0=x_t, scalar1=mv[:, 0:1], scalar2=rstd,
            op0=mybir.AluOpType.subtract, op1=mybir.AluOpType.mult,
        )
        return out_t

    def transpose_cast(h_ts, g_sb, b_sb, hT):
        """PE-transpose 4 subtiles of h [128, E] into hT[:, k, :] (mdt),
        batching pairs of 128x128 transposes into one PSUM tile so the
        layernorm scale/bias fold costs one DVE op per [128, 256]."""
        for k in range(NET):
            for half in range(2):
                tp = ps_tr.tile([TS, 2 * TS], f32, name="tp")
                for q in range(2):
                    s = half * 2 + q
                    nc.tensor.transpose(
                        tp[:, q * TS:(q + 1) * TS],
                        h_ts[s][:, k * ET:(k + 1) * ET], id_sb,
                    )
                nc.vector.tensor_scalar(
                    out=hT[:, k, half * 2 * TS:(half + 1) * 2 * TS], in0=tp,
                    scalar1=g_sb[:, k, :], scalar2=b_sb[:, k, :],
                    op0=mybir.AluOpType.mult, op1=mybir.AluOpType.add,
                )

    # =====================================================================
    # Phase 1: LN1 + transpose + QKV per chunk
    # =====================================================================
    for c in range(NTC):
        hT = hT_pool.tile([ET, NET, TC], mdt, name="hT")
        h_ts = []
        for s in range(NSUB):
            r0 = c * TC + s * TS
            x_t = xt_pool.tile([128, E], f32, name="x_t", bufs=5)
            nc.sync.dma_start(out=x_t, in_=xr[r0:r0 + TS, :])
            h_ts.append(layer_norm(x_t, eps1_sb))
        transpose_cast(h_ts, ln_sb["ln1g"], ln_sb["ln1b"], hT)
        for dd in range(NDT):
            for w_sb, dst in ((wq_sb, qT_c[c]), (wk_sb, kT_c[c])):
                ps = ps_mm.tile([128, TC], f32, name="ps_qk", tag="mm")
                for k in range(NET):
                    nc.tensor.matmul(
                        ps, mc(w_sb[:, k, dd * 128:(dd + 1) * 128]),
                        mc(hT[:, k, :]),
                        start=(k == 0), stop=(k == NET - 1),
                    )
                nc.vector.tensor_copy(dst[:, dd, :], ps)
        for s in range(NSUB):
            ps = ps_mm.tile([128, DSL], f32, name="ps_v", tag="mm")
            for k in range(NET):
                nc.tensor.matmul(
                    ps, mc(hT[:, k, s * TS:(s + 1) * TS]), mc(wv_sb[:, k, :]),
                    start=(k == 0), stop=(k == NET - 1),
                )
            nc.vector.tensor_copy(
                vt_c[c][:, s, :, 0:HS],
                ps.rearrange("p (h d) -> p h d", h=HPC),
            )
            nc.gpsimd.memset(vt_c[c][:, s, :, HS:HS + 1], 1.0)

    # =====================================================================
    # Phase 2: attention + proj partials, chunks 0, 2, 1, 3;
    # pair ReduceScatter after {0, 2} and after {1, 3}.
    # =====================================================================
    def attention_chunk(c):
        nkt = (c + 1) * NSUB
        attTs = []
        for pr in range(NDT):  # head pair = d-tile
            av_ps = [ps_av.tile([128, TC], f32, name="avp") for _ in range(2)]

            def do_scores(i):
                pts = []
                for hh in range(2):
                    h0 = hh * HS
                    sc = ps_sc.tile([TS, TC], f32, name="sc")
                    nc.tensor.matmul(
                        sc,
                        mc(kT_c[i // NSUB][h0:h0 + HS, pr,
                                           (i % NSUB) * TS:(i % NSUB + 1) * TS]),
                        mc(qT_c[c][h0:h0 + HS, pr, :]),
                        start=True, stop=True,
                    )
                    pt = pt_pool.tile([TS, TC], mdt, name="pt")
                    nc.scalar.activation(out=pt, in_=sc, func=AF.Exp, scale=SCALE)
                    m = i - c * NSUB
                    if m >= 0:
                        # diagonal tile: zero t_k > t_q; columns beyond
                        # (m+1)*TS are fully causal already
                        w = (m + 1) * TS
                        nc.vector.tensor_mul(
                            pt[:, 0:w], pt[:, 0:w], mask_sb[:, m, 0:w]
                        )
                    pts.append(pt)
                return pts

            def do_avs(i, pts):
                for hh in range(2):
                    nc.tensor.matmul(
                        av_ps[hh][0:HS + 1, :],
                        mc(vt_c[i // NSUB][:, i % NSUB, pr * 2 + hh, :]),
                        mc(pts[hh]),
                        start=(i == 0), stop=(i == nkt - 1),
                    )

            # stagger: emit scores+exp for a group of 3 t_k tiles, then the
            # av matmuls of the previous group, so PE never waits on exp.
            G = 3
            prev = []
            for g0 in range(0, nkt, G):
                cur = [(i, do_scores(i)) for i in range(g0, min(g0 + G, nkt))]
                for i, pts in prev:
                    do_avs(i, pts)
                prev = cur
            for i, pts in prev:
                do_avs(i, pts)

            # --- copy the accumulators out (releases PSUM for the next
            # pair), then denominators: transpose the two [1, 512] rows into
            # one [128, 8] column tile, one cheap wide reciprocal, and a
            # per-head DRAM bounce to broadcast across the 64 partitions.
            av_sb = [avs_pool.tile([HS + 1, TC], mdt, name="av_sb")
                     for _ in range(2)]
            for hh in range(2):
                nc.vector.tensor_copy(av_sb[hh][0:HS + 1, :],
                                      av_ps[hh][0:HS + 1, :])
            rbt_ps = ps_tr.tile([128, 2 * NSUB, 2], mdt, name="rbt_ps", tag="tp")
            with nc.allow_low_precision(reason="bf16 softmax denominators"):
                for hh in range(2):
                    for j in range(NSUB):
                        nc.tensor.transpose(
                            rbt_ps[:, hh * NSUB + j, 0:1],
                            av_sb[hh][HS:HS + 1, j * TS:(j + 1) * TS],
                            ones65[HS:HS + 1, :],
                        )
            rbt_sb = rbt_pool.tile([128, 2 * NSUB], mdt, name="rbt_sb")
            with nc.allow_low_precision(reason="bf16 softmax denominators"):
                nc.vector.reciprocal(out=rbt_sb, in_=rbt_ps[:, :, 0])
            attT = attT_pool.tile([128, TC], mdt, name="attT")
            for hh in range(2):
                dd_t = den_dram_pool.tile([1, TC], mdt, name="dd_t")
                for j in range(NSUB):
                    nc.sync.dma_start(
                        out=dd_t[:, j * TS:(j + 1) * TS],
                        in_=rbt_sb[:, hh * NSUB + j:hh * NSUB + j + 1],
                    )
                rb = rb_pool.tile([HS, TC], mdt, name="rb")
                nc.sync.dma_start(out=rb, in_=dd_t.to_broadcast((HS, TC)))
                nc.vector.tensor_mul(
                    attT[hh * HS:(hh + 1) * HS, :], av_sb[hh][0:HS, :], rb
                )
            attTs.append(attT)
        return attTs

    def proj_chunk(c, attTs):
        # ar_in layout: [reduce#, pair-rank-shard, TC, E] so each
        # ReduceScatter input is contiguous: chunk c -> region c%2, slot c//2.
        for s in range(NSUB):
            r0 = c * TC + s * TS
            w0 = (c % 2) * (2 * TC) + (c // 2) * TC + s * TS
            xr_t = xt_pool.tile([128, E], f32, name="xr_t")
            nc.sync.dma_start(out=xr_t, in_=xr[r0:r0 + TS, :])
            part = stage_pool.tile([128, E], f32, name="part", tag="stg", bufs=4)
            for n in range(2):
                ps = ps_mm.tile([128, TC], f32, name="ps_pr", tag="mm")
                for dd in range(NDT):
                    nc.tensor.matmul(
                        ps, mc(attTs[dd][:, s * TS:(s + 1) * TS]),
                        mc(wp_sb[:, dd, n * TC:(n + 1) * TC]),
                        start=(dd == 0), stop=False,
                    )
                nc.tensor.matmul(
                    ps, mc(ones1), mc(bp_sb[:, n * TC:(n + 1) * TC]),
                    start=False, stop=True,
                )
                nc.vector.tensor_add(
                    part[:, n * TC:(n + 1) * TC], ps, xr_t[:, n * TC:(n + 1) * TC]
                )
            nc.sync.dma_start(out=ar_in[w0:w0 + TS, :], in_=part)
            if DEBUG:
                nc.sync.dma_start(out=io["dbg"][r0:r0 + TS, :], in_=part)

    for c in (0, 2, 1, 3):
        attTs = attention_chunk(c)
        proj_chunk(c, attTs)
        if c == 1:
            nc.gpsimd.collective_compute(
                "ReduceScatter", mybir.AluOpType.add, replica_groups=PAIRS,
                ins=[ar_in[0:2 * TC, :]], outs=[rs_out[0]],
            )
    nc.gpsimd.collective_compute(
        "ReduceScatter", mybir.AluOpType.add, replica_groups=PAIRS,
        ins=[ar_in[2 * TC:4 * TC, :]], outs=[rs_out[1]],
    )

    # =====================================================================
    # Phase 3: LN2 + FFN + residual on this core's half (2 local chunks)
    # =====================================================================
    for lc in range(2):
        h2T = hT_pool.tile([ET, NET, TC], mdt, name="h2T")
        x2_ts = []
        h2_ts = []
        for s in range(NSUB):
            x2_t = stage_pool.tile([128, E], f32, name="x2_t", tag="stg", bufs=4)
            nc.sync.dma_start(out=x2_t, in_=rs_out[lc][s * TS:(s + 1) * TS, :])
            x2_ts.append(x2_t)
            h2_t = xt_pool.tile([128, E], f32, name="h2_t", tag="x_t", bufs=5)
            layer_norm(x2_t, eps2_sb, out_t=h2_t)
            h2_ts.append(h2_t)
        transpose_cast(h2_ts, ln_sb["ln2g"], ln_sb["ln2b"], h2T)
        f1 = f1_pool.tile([FFN + 1, TC], mdt, name="f1")
        nc.vector.memset(f1, 1.0)  # row FFN stays 1.0 (b2 matmul row)
        ps_f = ps_mm.tile([FFN, TC], f32, name="ps_f", tag="mm")
        for k in range(NET):
            nc.tensor.matmul(
                ps_f, mc(w1_sb[:, k, :]), mc(h2T[:, k, :]),
                start=(k == 0), stop=(k == NET - 1),
            )
        nc.scalar.activation(
            out=f1[0:FFN, :], in_=ps_f, func=AF.Relu, bias=b1_sb, scale=1.0
        )
        for s in range(NSUB):
            o_t = out_pool.tile([128, E], f32, name="o_t")
            for n in range(2):
                ps = ps_mm.tile([128, TC], f32, name="ps_o", tag="mm")
                nc.tensor.matmul(
                    ps, mc(f1[:, s * TS:(s + 1) * TS]),
                    mc(w2_sb[:, n * TC:(n + 1) * TC]),
                    start=True, stop=True,
                )
                nc.vector.tensor_add(
                    o_t[:, n * TC:(n + 1) * TC], ps,
                    x2_ts[s][:, n * TC:(n + 1) * TC],
                )
            r0 = lc * TC + s * TS
            nc.sync.dma_start(out=out[r0:r0 + TS, :], in_=o_t)


# =========================================================================
# Host side
# =========================================================================
def _make_masks(np_mdt):
    # masks[p, d, f] = 1 iff t_k <= t_q for the diagonal block at offset d,
    # i.e. f >= 128*d + p  (t_k = 128*i + p, t_q = 512*c + f, i = 4*c + d)
    m = np.zeros((TS, NSUB, TC), dtype=np.float32)
    for d in range(NSUB):
        for p in range(TS):
            m[p, d, d * TS + p:] = 1.0
    return m.astype(np_mdt)


_NC_CACHE = {}
RUN_KWARGS = {}      # test harness may set {"trace": True} for profiling
LAST_RESULT = None   # BassKernelResults of the most recent run


def kernel(x, wq, wk, wv, w_proj, b_proj, w1, b1, w2, b2, ln1_g, ln1_b, ln2_g,
           ln2_b):
    mode = MM_MODE
    np_mdt = _np_mdt(mode)
    if mode not in _NC_CACHE:
        _NC_CACHE[mode] = build(mode)
    nc = _NC_CACHE[mode]

    x = np.asarray(x, np.float32)
    masks = _make_masks(np_mdt)
    identity = np.eye(TS, dtype=np.float32)
    w2e = np.concatenate([np.asarray(w2, np.float32),
                          np.asarray(b2, np.float32)[None, :]], axis=0)
    in_maps = []
    for core in range(NCORE):
        b, g = core // 2, core % 2
        sl = slice(g * DSL, (g + 1) * DSL)
        in_maps.append({
            "xr": 0.5 * x[b],
            "wq": np.asarray(wq, np.float32)[:, sl].astype(np_mdt),
            "wk": np.asarray(wk, np.float32)[:, sl].astype(np_mdt),
            "wv": np.asarray(wv, np.float32)[:, sl].astype(np_mdt),
            "wp": np.asarray(w_proj, np.float32)[sl, :].astype(np_mdt),
            "bp": (0.5 * np.asarray(b_proj, np.float32))[None, :].astype(np_mdt),
            "w1": np.asarray(w1, np.float32).astype(np_mdt),
            "w2e": w2e.astype(np_mdt),
            "b1": np.asarray(b1, np.float32)[:, None],
            "ln1g": np.asarray(ln1_g, np.float32)[:, None],
            "ln1b": np.asarray(ln1_b, np.float32)[:, None],
            "ln2g": np.asarray(ln2_g, np.float32)[:, None],
            "ln2b": np.asarray(ln2_b, np.float32)[:, None],
            "masks": masks,
            "ident": identity,
        })
    global LAST_RESULT
    res = run_bass_kernel_spmd(nc, in_maps, list(range(NCORE)), **RUN_KWARGS)
    LAST_RESULT = res
    outp = np.empty((B, T, E), np.float32)
    for core in range(NCORE):
        b, g = core // 2, core % 2
        outp[b, g * (T // 2):(g + 1) * (T // 2), :] = res.results[core]["out"]
    return outp



# revision 3
# speedup vs baseline: 2.3611x; 2.3611x over previous
"""Trainium2 Bass kernel: pre-LN transformer block (B=4, T=2048, E=1024, H=16, FFN=100).

Sharding (8 NeuronCores): core 2b+g handles batch b, head-group g (8 of 16 heads,
i.e. a 512-wide slice of the QKV output dim / proj input dim).  Both cores of a
pair compute attention + proj partials for all 2048 tokens of their batch; a
per-chunk (512-token) pair ReduceScatter in bf16 combines the pure proj partials
and hands each core 256 tokens of the chunk, on which it runs the residual add
(x + b_proj folded host-side into the per-core x_own input), LN2 + FFN, and
writes its [4, 256, 1024] output shard.  The four chunk-RS calls are issued as
soon as each chunk's proj is done so they overlap the next chunks' attention;
per-chunk FFN work is interleaved between attention chunks the same way.

Attention layout: scores are computed transposed, S^T[t_k, t_q] = k^T.T @ q^T,
with q^T/k^T in [head_dim, token] layout (from PE-transposed LN output).  The
two heads of a d-tile pair occupy partitions 0-63 / 64-127, and their score
matmuls write the two halves of one 2-bank PSUM tile so a single ScalarE exp
(1/sqrt(E) scale folded in) covers both.  Causal masking multiplies diagonal
tiles by precomputed patterns; the softmax denominator comes from a ones column
appended to V, is reciprocated on VectorE, and is broadcast across the head's
64 partitions with a K=1 ones-matmul into PSUM (no DRAM bounce).  LayerNorm
rsqrt is computed as exp(-0.5*ln(var+eps)) so the whole kernel uses a single
ScalarE table set (natural_log_exp_and_others).
"""

from contextlib import ExitStack

import numpy as np
import ml_dtypes

import concourse.bass as bass
import concourse.mybir as mybir
import concourse.tile as tile
from concourse.bass_utils import run_bass_kernel_spmd
from concourse.vector_clock import ScopedClock


class SplitDrainTC(tile.TileContext):
    """Works around a walrus codegen limit: an SP CTRL instruction may carry
    only one sync wait, so the kernel-tail drain's waits are split onto
    preceding single-wait nops."""

    def _drain_and_barrier(self, tick_clock, wait_clock):
        probe = self.nc.sync.nop(nofuse=True)
        wait_clock.add_sem_waits(
            probe.ins, ScopedClock({None: tick_clock.global_clock})
        )
        si = probe.ins.sync_info
        waits = list(si.on_wait) if si is not None else []
        if len(waits) > 1:
            si.on_wait = [waits[0]]
            for w in waits[1:]:
                n2 = self.nc.sync.nop(nofuse=True)
                n2.ins.sync_info = mybir.SyncInfo(on_wait=[w], on_update=[])
        self.nc.sync.drain()
        self.nc.all_engine_barrier()
        popped = self.nc._tile_sem_poison_stack.pop()
        assert popped is self._sem_poison
        self.nc.clear_and_free_semaphores(list(self.sems.allocated().values()))
        self.nc.all_engine_barrier()

B, T, E, H, HS, FFN = 4, 2048, 1024, 16, 64, 100
EPS = 1e-5
NCORE = 8
TC = 512            # token chunk
NTC = T // TC       # 4
TS = 128            # token subtile
NSUB = TC // TS     # 4
ET = 128            # embed tile
NET = E // ET       # 8
DSL = E // 2        # per-core qkv output slice (8 heads * 64)
NDT = DSL // 128    # 4 d-tiles (2 heads each)
HPC = H // 2        # 8 heads per core
SCALE = float(E) ** -0.5
PAIRS = [[0, 1], [2, 3], [4, 5], [6, 7]]

MM_MODE = "bf16"    # "bf16" | "f32r" | "f32"
AF = mybir.ActivationFunctionType


def _mdt(mode):
    return mybir.dt.bfloat16 if mode == "bf16" else mybir.dt.float32


def _np_mdt(mode):
    return ml_dtypes.bfloat16 if mode == "bf16" else np.float32


def build(mode=MM_MODE):
    f32 = mybir.dt.float32
    mdt = _mdt(mode)

    def mc(ap):
        """Cast an AP for use as a matmul operand."""
        if mode == "f32r":
            return ap.bitcast(mybir.dt.float32r)
        return ap

    nc = bass.Bass(num_devices=NCORE)

    io = {}

    def param(name, shape, dtype):
        io[name] = nc.declare_dram_parameter(name, shape, dtype, isOutput=False)

    param("x", [T, E], f32)
    param("x_own", [NTC, TC // 2, E], f32)   # own scattered rows, + b_proj
    param("wq", [E, DSL], mdt)
    param("wk", [E, DSL], mdt)
    param("wv", [E, DSL], mdt)
    param("wp", [DSL, E], mdt)
    param("w1", [E, FFN], mdt)
    param("w2e", [FFN + 1, E], mdt)    # w2 with b2 as the extra last row
    param("b1", [FFN, 1], f32)
    param("ln1g", [E, 1], f32)
    param("ln1b", [E, 1], f32)
    param("ln2g", [E, 1], f32)
    param("ln2b", [E, 1], f32)
    param("masks", [TS, NSUB, TC], mdt)
    param("ident", [TS, TS], f32)
    io["out"] = nc.declare_dram_parameter(
        "out", [NTC, TC // 2, E], f32, isOutput=True
    )

    with SplitDrainTC(nc) as tc:
        with ExitStack() as ctx:
            _build_tile(ctx, tc, nc, mode, mdt, f32, mc, io)
    _split_waits(nc)
    return nc


def _split_waits(nc, maxw=1):
    """walrus codegen accepts a limited number of sync waits per instruction;
    move the excess onto same-engine NoOps inserted just before."""
    import bass_rust
    n = 0
    for f in nc.m.functions:
        for b in f.blocks:
            new = []
            for inst in b.instructions:
                si = inst.sync_info
                # fixed-length ISA instructions can't carry waits at all
                cap = 0 if isinstance(inst, bass_rust.InstISA) else maxw
                if si is not None and len(si.on_wait) > cap:
                    waits = list(si.on_wait)
                    keep = waits[-cap:] if cap else []
                    excess = waits[:-cap] if cap else waits
                    for w in excess:
                        nop = mybir.InstNoOp(
                            name=f"{inst.name}-wsplit{n}", engine=inst.engine
                        )
                        nop.bass_nofuse = True
                        n += 1
                        nop.sync_info = mybir.SyncInfo(
                            on_wait=[w], on_update=[]
                        )
                        new.append(nop)
                    si.on_wait = keep
                new.append(inst)
            if n:
                b.instructions = new


def _build_tile(ctx, tc, nc, mode, mdt, f32, mc, io):
    x, out = io["x"], io["out"]

    def pool(name, bufs, space="SBUF"):
        return ctx.enter_context(tc.tile_pool(name=name, bufs=bufs, space=space))

    # ---- internal DRAM: per-chunk proj-partial RS buffers ----
    dram = pool("dram", 1, space="DRAM")
    ar_c = [dram.tile([TC, E], mdt, name=f"ar{c}") for c in range(NTC)]
    rs_c = [dram.tile([TC // 2, E], mdt, name=f"rs{c}") for c in range(NTC)]

    # ---- persistent SBUF: weights & constants ----
    wpool = pool("weights", 1)
    wq_sb = wpool.tile([ET, NET, DSL], mdt, name="wq_sb")
    wk_sb = wpool.tile([ET, NET, DSL], mdt, name="wk_sb")
    wv_sb = wpool.tile([ET, NET, DSL], mdt, name="wv_sb")
    nc.sync.dma_start(out=wq_sb, in_=io["wq"].rearrange("(k p) d -> p k d", p=ET))
    nc.sync.dma_start(out=wk_sb, in_=io["wk"].rearrange("(k p) d -> p k d", p=ET))
    nc.sync.dma_start(out=wv_sb, in_=io["wv"].rearrange("(k p) d -> p k d", p=ET))
    wp_sb = wpool.tile([128, NDT, E], mdt, name="wp_sb")
    nc.sync.dma_start(out=wp_sb, in_=io["wp"].rearrange("(k p) d -> p k d", p=128))
    ones_row = wpool.tile([1, HS], mdt, name="ones_row")
    nc.vector.memset(ones_row, 1.0)
    w1_sb = wpool.tile([ET, NET, FFN], mdt, name="w1_sb")
    nc.sync.dma_start(out=w1_sb, in_=io["w1"].rearrange("(k p) d -> p k d", p=ET))
    w2_sb = wpool.tile([FFN + 1, E], mdt, name="w2_sb")
    nc.sync.dma_start(out=w2_sb, in_=io["w2e"][:])
    b1_sb = wpool.tile([FFN, 1], f32, name="b1_sb")
    nc.sync.dma_start(out=b1_sb, in_=io["b1"][:])
    ln_sb = {}
    for nm in ("ln1g", "ln1b", "ln2g", "ln2b"):
        ln_sb[nm] = wpool.tile([ET, NET, 1], f32, name=nm + "_sb")
        nc.sync.dma_start(
            out=ln_sb[nm], in_=io[nm].rearrange("(k p) o -> p k o", p=ET)
        )
    mask_sb = wpool.tile([TS, NSUB, TC], mdt, name="mask_sb")
    nc.sync.dma_start(out=mask_sb, in_=io["masks"][:])
    id_sb = wpool.tile([TS, TS], f32, name="id_sb")
    nc.sync.dma_start(out=id_sb, in_=io["ident"][:])
    eps_sb = wpool.tile([128, 1], f32, name="eps_sb")
    nc.vector.memset(eps_sb, EPS)

    # ---- persistent SBUF: per-chunk K^T, V(+ones), Q^T ----
    kv = pool("kv", 1)
    kT_c = [kv.tile([128, NDT, TC], mdt, name=f"kT{c}") for c in range(NTC)]
    vt_c = [kv.tile([128, NSUB, HPC, HS + 1], mdt, name=f"vt{c}")
            for c in range(NTC)]
    qT_c = [kv.tile([128, NDT, TC], mdt, name=f"qT{c}") for c in range(NTC)]

    # ---- working pools ----
    xt_pool = pool("xt", 5)        # [128, E] f32: x rows for LN1, h2 for LN2
    mv_pool = pool("mv", 3)
    hT_pool = pool("hT", 2)        # [128, NET, TC] bf16
    pt_pool = pool("pt", 6)        # [128, 2, TC] bf16 softmax tiles
    avs_pool = pool("avs", 3)      # [HS+1, 2, TC] bf16
    dr_pool = pool("dr", 3)        # [1, 2, TC] bf16 reciprocal denominators
    attT_pool = pool("attT", 6)    # [128, TC] bf16
    stage_pool = pool("stage", 4)  # [128, E] bf16: proj partials, rs loads
    x2_pool = pool("x2", 4)        # [128, E] f32: phase-3 residual rows
    f1_pool = pool("f1", 2)
    out_pool = pool("outp", 2)
    ps_mm = pool("ps_mm", 2, space="PSUM")   # [128, 512] (1 bank each)
    ps_sc = pool("ps_sc", 2, space="PSUM")   # [128, 2, 512] (2 banks each)
    ps_av = pool("ps_av", 2, space="PSUM")   # [HS+1, 512] (1 bank each)

    def layer_norm(x_t, out_t=None):
        """(x - mean) * rsqrt(var + eps); in-place unless out_t given.
        rsqrt is exp(-0.5*ln(var+eps)) to stay in one ScalarE table set."""
        stats = mv_pool.tile([128, 2, nc.vector.BN_STATS_DIM], f32, name="stats")
        xg = x_t.rearrange("p (s q) -> p s q", s=2)
        for s in range(2):
            nc.vector.bn_stats(out=stats[:, s, :], in_=xg[:, s, :])
        mv = mv_pool.tile([128, 2], f32, name="mv")
        nc.vector.bn_aggr(out=mv, in_=stats)
        rstd = mv_pool.tile([128, 1], f32, name="rstd")
        nc.scalar.activation(
            out=rstd, in_=mv[:, 1:2], func=AF.Ln, bias=eps_sb, scale=1.0
        )
        nc.scalar.activation(out=rstd, in_=rstd, func=AF.Exp, scale=-0.5)
        if out_t is None:
            out_t = x_t
        nc.vector.tensor_scalar(
            out=out_t, in0=x_t, scalar1=mv[:, 0:1], scalar2=rstd,
            op0=mybir.AluOpType.subtract, op1=mybir.AluOpType.mult,
        )
        return out_t

    def transpose_cast(h_ts, g_sb, b_sb, hT, width):
        """PE-transpose len(h_ts) subtiles of h [128, E] into hT[:, k, :]
        (bf16), batching all of them into one PSUM tile per e-tile so the
        layernorm scale/bias fold costs one DVE op per [128, width]."""
        nsub = len(h_ts)
        for k in range(NET):
            tp = ps_mm.tile([TS, nsub * TS], f32, name="tp", tag="mm")
            for s in range(nsub):
                nc.tensor.transpose(
                    tp[:, s * TS:(s + 1) * TS],
                    h_ts[s][:, k * ET:(k + 1) * ET], id_sb,
                )
            nc.vector.tensor_scalar(
                out=hT[:, k, 0:width], in0=tp,
                scalar1=g_sb[:, k, :], scalar2=b_sb[:, k, :],
                op0=mybir.AluOpType.mult, op1=mybir.AluOpType.add,
            )

    # =====================================================================
    # Phase 1: LN1 + transpose + QKV per chunk
    # =====================================================================
    def qkv_chunk(c):
        hT = hT_pool.tile([ET, NET, TC], mdt, name="hT")
        h_ts = []
        for s in range(NSUB):
            r0 = c * TC + s * TS
            x_t = xt_pool.tile([128, E], f32, name="x_t")
            nc.sync.dma_start(out=x_t, in_=x[r0:r0 + TS, :])
            h_ts.append(layer_norm(x_t))
        transpose_cast(h_ts, ln_sb["ln1g"], ln_sb["ln1b"], hT, TC)
        for dd in range(NDT):
            for w_sb, dst in ((wq_sb, qT_c[c]), (wk_sb, kT_c[c])):
                ps = ps_mm.tile([128, TC], f32, name="ps_qk", tag="mm")
                for k in range(NET):
                    nc.tensor.matmul(
                        ps, mc(w_sb[:, k, dd * 128:(dd + 1) * 128]),
                        mc(hT[:, k, :]),
                        start=(k == 0), stop=(k == NET - 1),
                    )
                nc.vector.tensor_copy(dst[:, dd, :], ps)
        for s in range(NSUB):
            ps = ps_mm.tile([128, DSL], f32, name="ps_v", tag="mm")
            for k in range(NET):
                nc.tensor.matmul(
                    ps, mc(hT[:, k, s * TS:(s + 1) * TS]), mc(wv_sb[:, k, :]),
                    start=(k == 0), stop=(k == NET - 1),
                )
            nc.vector.tensor_copy(
                vt_c[c][:, s, :, 0:HS],
                ps.rearrange("p (h d) -> p h d", h=HPC),
            )
            nc.gpsimd.memset(vt_c[c][:, s, :, HS:HS + 1], 1.0)

    # =====================================================================
    # Phase 2: attention + proj partials + per-chunk pair ReduceScatter
    # =====================================================================
    def finish_pair(av_sb, dr, attT):
        """Broadcast the reciprocal denominators across each head's 64
        partitions with a K=1 ones-matmul and normalize into attT (bf16)."""
        for hh in range(2):
            rb = ps_mm.tile([HS, TC], f32, name="rb", tag="mm")
            nc.tensor.matmul(
                rb, mc(ones_row), mc(dr[:, hh, :]), start=True, stop=True
            )
            nc.vector.tensor_mul(
                attT[hh * HS:(hh + 1) * HS, :], av_sb[0:HS, hh, :], rb
            )

    def attention_chunk(c):
        nkt = (c + 1) * NSUB
        attTs = []
        pending = None
        for pr in range(NDT):  # head pair = d-tile
            av_ps = [ps_av.tile([HS + 1, TC], f32, name="avp") for _ in range(2)]
            prev = None
            for i in range(nkt):
                sc2 = ps_sc.tile([TS, 2, TC], f32, name="sc2")
                for hh in range(2):
                    h0 = hh * HS
                    nc.tensor.matmul(
                        sc2[:, hh, :],
                        mc(kT_c[i // NSUB][h0:h0 + HS, pr,
                                           (i % NSUB) * TS:(i % NSUB + 1) * TS]),
                        mc(qT_c[c][h0:h0 + HS, pr, :]),
                        start=True, stop=True,
                    )
                pt2 = pt_pool.tile([TS, 2, TC], mdt, name="pt2")
                nc.scalar.activation(out=pt2, in_=sc2, func=AF.Exp, scale=SCALE)
                m = i - c * NSUB
                if m >= 0:
                    # diagonal tile: zero t_k > t_q; columns beyond
                    # (m+1)*TS are fully causal already
                    w = (m + 1) * TS
                    for hh in range(2):
                        nc.vector.tensor_mul(
                            pt2[:, hh, 0:w], pt2[:, hh, 0:w], mask_sb[:, m, 0:w]
                        )
                if prev is not None:
                    pi, ppt = prev
                    for hh in range(2):
                        nc.tensor.matmul(
                            av_ps[hh],
                            mc(vt_c[pi // NSUB][:, pi % NSUB, pr * 2 + hh, :]),
                            mc(ppt[:, hh, :]),
                            start=(pi == 0), stop=False,
                        )
                if i == 1 and pending is not None:
                    finish_pair(*pending)
                    pending = None
                prev = (i, pt2)
            pi, ppt = prev
            for hh in range(2):
                nc.tensor.matmul(
                    av_ps[hh],
                    mc(vt_c[pi // NSUB][:, pi % NSUB, pr * 2 + hh, :]),
                    mc(ppt[:, hh, :]),
                    start=(pi == 0), stop=True,
                )
            av_sb = avs_pool.tile([HS + 1, 2, TC], mdt, name="av_sb")
            for hh in range(2):
                nc.vector.tensor_copy(av_sb[:, hh, :], av_ps[hh])
            dr = dr_pool.tile([1, 2, TC], mdt, name="dr")
            with nc.allow_low_precision(reason="bf16 softmax denominators"):
                nc.vector.reciprocal(out=dr, in_=av_sb[HS:HS + 1, :, :])
            attT = attT_pool.tile([128, TC], mdt, name="attT")
            attTs.append(attT)
            if pending is not None:
                finish_pair(*pending)
            pending = (av_sb, dr, attT)
        finish_pair(*pending)
        return attTs

    def proj_chunk(c, attTs):
        for s in range(NSUB):
            part = stage_pool.tile([128, E], mdt, name="part", tag="stg")
            for n in range(2):
                ps = ps_mm.tile([128, TC], f32, name="ps_pr", tag="mm")
                for dd in range(NDT):
                    nc.tensor.matmul(
                        ps, mc(attTs[dd][:, s * TS:(s + 1) * TS]),
                        mc(wp_sb[:, dd, n * TC:(n + 1) * TC]),
                        start=(dd == 0), stop=(dd == NDT - 1),
                    )
                nc.vector.tensor_copy(part[:, n * TC:(n + 1) * TC], ps)
            nc.sync.dma_start(out=ar_c[c][s * TS:(s + 1) * TS, :], in_=part)
        nc.gpsimd.collective_compute(
            "ReduceScatter", mybir.AluOpType.add, replica_groups=PAIRS,
            ins=[ar_c[c][:]], outs=[rs_c[c]],
        )

    # =====================================================================
    # Phase 3: residual + LN2 + FFN on this core's 256-token shard of chunk c
    # =====================================================================
    def ffn_chunk(c):
        x2_ts = []
        h2_ts = []
        for s in range(2):
            rs_sb = stage_pool.tile([128, E], mdt, name="rs_sb", tag="stg")
            nc.sync.dma_start(out=rs_sb, in_=rs_c[c][s * TS:(s + 1) * TS, :])
            x2_t = x2_pool.tile([128, E], f32, name="x2_t")
            nc.sync.dma_start(out=x2_t, in_=io["x_own"][c, s * TS:(s + 1) * TS, :])
            nc.vector.tensor_add(x2_t, x2_t, rs_sb)
            x2_ts.append(x2_t)
            h2_t = xt_pool.tile([128, E], f32, name="h2_t", tag="x_t")
            layer_norm(x2_t, out_t=h2_t)
            h2_ts.append(h2_t)
        h2T = hT_pool.tile([ET, NET, TC // 2], mdt, name="h2T")
        transpose_cast(h2_ts, ln_sb["ln2g"], ln_sb["ln2b"], h2T, TC // 2)
        f1 = f1_pool.tile([FFN + 1, TC // 2], mdt, name="f1")
        nc.vector.memset(f1, 1.0)  # row FFN stays 1.0 (b2 matmul row)
        ps_f = ps_mm.tile([FFN, TC // 2], f32, name="ps_f", tag="mm")
        for k in range(NET):
            nc.tensor.matmul(
                ps_f, mc(w1_sb[:, k, :]), mc(h2T[:, k, :]),
                start=(k == 0), stop=(k == NET - 1),
            )
        nc.scalar.activation(
            out=f1[0:FFN, :], in_=ps_f, func=AF.Relu, bias=b1_sb, scale=1.0
        )
        for s in range(2):
            o_t = out_pool.tile([128, E], f32, name="o_t")
            for n in range(2):
                ps = ps_mm.tile([128, TC], f32, name="ps_o", tag="mm")
                nc.tensor.matmul(
                    ps, mc(f1[:, s * TS:(s + 1) * TS]),
                    mc(w2_sb[:, n * TC:(n + 1) * TC]),
                    start=True, stop=True,
                )
                nc.vector.tensor_add(
                    o_t[:, n * TC:(n + 1) * TC], ps,
                    x2_ts[s][:, n * TC:(n + 1) * TC],
                )
            nc.sync.dma_start(out=out[c, s * TS:(s + 1) * TS, :], in_=o_t)

    # ---- schedule: qkv all chunks; attention+proj+RS per chunk with the
    # previous chunk's FFN interleaved so it overlaps this chunk's RS wait ----
    for c in range(NTC):
        qkv_chunk(c)
    for c in range(NTC):
        attTs = attention_chunk(c)
        proj_chunk(c, attTs)
        if c >= 1:
            ffn_chunk(c - 1)
    ffn_chunk(NTC - 1)


# =========================================================================
# Host side
# =========================================================================
def _make_masks(np_mdt):
    # masks[p, d, f] = 1 iff t_k <= t_q for the diagonal block at offset d,
    # i.e. f >= 128*d + p  (t_k = 128*i + p, t_q = 512*c + f, i = 4*c + d)
    m = np.zeros((TS, NSUB, TC), dtype=np.float32)
    for d in range(NSUB):
        for p in range(TS):
            m[p, d, d * TS + p:] = 1.0
    return m.astype(np_mdt)


_NC_CACHE = {}
RUN_KWARGS = {}      # test harness may set {"trace": True} for profiling
LAST_RESULT = None   # BassKernelResults of the most recent run


def kernel(x, wq, wk, wv, w_proj, b_proj, w1, b1, w2, b2, ln1_g, ln1_b, ln2_g,
           ln2_b):
    mode = MM_MODE
    np_mdt = _np_mdt(mode)
    if mode not in _NC_CACHE:
        _NC_CACHE[mode] = build(mode)
    nc = _NC_CACHE[mode]

    x = np.asarray(x, np.float32)
    bp = np.asarray(b_proj, np.float32)
    masks = _make_masks(np_mdt)
    identity = np.eye(TS, dtype=np.float32)
    w2e = np.concatenate([np.asarray(w2, np.float32),
                          np.asarray(b2, np.float32)[None, :]], axis=0)
    in_maps = []
    for core in range(NCORE):
        b, g = core // 2, core % 2
        sl = slice(g * DSL, (g + 1) * DSL)
        # rows this core owns after the per-chunk pair ReduceScatter
        x_own = np.stack(
            [x[b, c * TC + g * (TC // 2):c * TC + (g + 1) * (TC // 2), :]
             for c in range(NTC)]
        ) + bp[None, None, :]
        in_maps.append({
            "x": x[b],
            "x_own": x_own,
            "wq": np.asarray(wq, np.float32)[:, sl].astype(np_mdt),
            "wk": np.asarray(wk, np.float32)[:, sl].astype(np_mdt),
            "wv": np.asarray(wv, np.float32)[:, sl].astype(np_mdt),
            "wp": np.asarray(w_proj, np.float32)[sl, :].astype(np_mdt),
            "w1": np.asarray(w1, np.float32).astype(np_mdt),
            "w2e": w2e.astype(np_mdt),
            "b1": np.asarray(b1, np.float32)[:, None],
            "ln1g": np.asarray(ln1_g, np.float32)[:, None],
            "ln1b": np.asarray(ln1_b, np.float32)[:, None],
            "ln2g": np.asarray(ln2_g, np.float32)[:, None],
            "ln2b": np.asarray(ln2_b, np.float32)[:, None],
            "masks": masks,
            "ident": identity,
        })
    global LAST_RESULT
    res = run_bass_kernel_spmd(nc, in_maps, list(range(NCORE)), **RUN_KWARGS)
    LAST_RESULT = res
    outp = np.empty((B, T, E), np.float32)
    for core in range(NCORE):
        b, g = core // 2, core % 2
        o = res.results[core]["out"]
        for c in range(NTC):
            r0 = c * TC + g * (TC // 2)
            outp[b, r0:r0 + TC // 2, :] = o[c]
    return outp


# revision 18
# speedup vs baseline: 2.8263x; 1.1971x over previous
"""Trainium2 Bass kernel: pre-LN transformer block (B=4, T=2048, E=1024, H=16, FFN=100).

Sharding (8 NeuronCores): core 2b+g handles batch b, head-group g (8 of 16 heads,
i.e. a 512-wide slice of the QKV output dim / proj input dim).  Both cores of a
pair compute attention + proj partials for all 2048 tokens of their batch; a
per-chunk (512-token) pair ReduceScatter in bf16 combines the pure proj partials
and hands each core 256 tokens of the chunk, on which it runs the residual add
(x + b_proj folded host-side into the per-core x_own input), LN2 + FFN, and
writes its [4, 256, 1024] output shard.  The four chunk-RS calls are issued as
soon as each chunk's proj is done so they overlap the next chunks' attention;
per-chunk FFN work is interleaved between attention chunks the same way.

Attention layout: scores are computed transposed, S^T[t_k, t_q] = k^T.T @ q^T,
with q^T/k^T in [head_dim, token] layout (from PE-transposed LN output).  The
two heads of a d-tile pair occupy partitions 0-63 / 64-127, and their score
matmuls write the two halves of one 2-bank PSUM tile so a single ScalarE exp
(1/sqrt(E) scale folded in) covers both.  Causal masking multiplies diagonal
tiles by precomputed patterns; the softmax denominator comes from a ones column
appended to V, is reciprocated on VectorE, and is broadcast across the head's
64 partitions with a K=1 ones-matmul into PSUM (no DRAM bounce).  LayerNorm
rsqrt is computed as exp(-0.5*ln(var+eps)) so the whole kernel uses a single
ScalarE table set (natural_log_exp_and_others).
"""

from contextlib import ExitStack

import numpy as np
import ml_dtypes

import concourse.bass as bass
import concourse.mybir as mybir
import concourse.tile as tile
from concourse.bass_utils import run_bass_kernel_spmd
from concourse.vector_clock import ScopedClock


class SplitDrainTC(tile.TileContext):
    """Works around a walrus codegen limit: an SP CTRL instruction may carry
    only one sync wait, so the kernel-tail drain's waits are split onto
    preceding single-wait nops."""

    def _drain_and_barrier(self, tick_clock, wait_clock):
        probe = self.nc.sync.nop(nofuse=True)
        wait_clock.add_sem_waits(
            probe.ins, ScopedClock({None: tick_clock.global_clock})
        )
        si = probe.ins.sync_info
        waits = list(si.on_wait) if si is not None else []
        if len(waits) > 1:
            si.on_wait = [waits[0]]
            for w in waits[1:]:
                n2 = self.nc.sync.nop(nofuse=True)
                n2.ins.sync_info = mybir.SyncInfo(on_wait=[w], on_update=[])
        self.nc.sync.drain()
        self.nc.all_engine_barrier()
        popped = self.nc._tile_sem_poison_stack.pop()
        assert popped is self._sem_poison
        self.nc.clear_and_free_semaphores(list(self.sems.allocated().values()))
        self.nc.all_engine_barrier()

B, T, E, H, HS, FFN = 4, 2048, 1024, 16, 64, 100
EPS = 1e-5
NCORE = 8
TC = 512            # token chunk
NTC = T // TC       # 4
TS = 128            # token subtile
NSUB = TC // TS     # 4
ET = 128            # embed tile
NET = E // ET       # 8
DSL = E // 2        # per-core qkv output slice (8 heads * 64)
NDT = DSL // 128    # 4 d-tiles (2 heads each)
HPC = H // 2        # 8 heads per core
SCALE = float(E) ** -0.5
PAIRS = [[0, 1], [2, 3], [4, 5], [6, 7]]

MM_MODE = "bf16"    # "bf16" | "f32r" | "f32"
AF = mybir.ActivationFunctionType


def _mdt(mode):
    return mybir.dt.bfloat16 if mode == "bf16" else mybir.dt.float32


def _np_mdt(mode):
    return ml_dtypes.bfloat16 if mode == "bf16" else np.float32


def build(mode=MM_MODE):
    f32 = mybir.dt.float32
    mdt = _mdt(mode)

    def mc(ap):
        """Cast an AP for use as a matmul operand."""
        if mode == "f32r":
            return ap.bitcast(mybir.dt.float32r)
        return ap

    nc = bass.Bass(num_devices=NCORE)

    io = {}

    def param(name, shape, dtype):
        io[name] = nc.declare_dram_parameter(name, shape, dtype, isOutput=False)

    param("x", [T, E], f32)
    param("x_own", [NTC, TC // 2, E], f32)   # own scattered rows, + b_proj
    param("wq", [E, DSL], mdt)
    param("wk", [E, DSL], mdt)
    param("wv", [E, DSL], mdt)
    param("wp", [DSL, E], mdt)
    param("w1", [E, FFN], mdt)
    param("w2e", [FFN + 1, E], mdt)    # w2 with b2 as the extra last row
    param("b1", [FFN, 1], f32)
    param("ln1g", [E, 1], f32)
    param("ln1b", [E, 1], f32)
    param("ln2g", [E, 1], f32)
    param("ln2b", [E, 1], f32)
    param("masks", [TS, NSUB, TC], mdt)
    param("ident", [TS, TS], mdt)
    io["out"] = nc.declare_dram_parameter(
        "out", [NTC, TC // 2, E], f32, isOutput=True
    )

    with SplitDrainTC(nc) as tc:
        with ExitStack() as ctx:
            _build_tile(ctx, tc, nc, mode, mdt, f32, mc, io)
    _split_waits(nc)
    return nc


def _split_waits(nc, maxw=1):
    """walrus codegen accepts a limited number of sync waits per instruction;
    move the excess onto same-engine NoOps inserted just before."""
    import bass_rust
    n = 0
    for f in nc.m.functions:
        for b in f.blocks:
            new = []
            for inst in b.instructions:
                si = inst.sync_info
                # fixed-length ISA instructions can't carry waits at all
                cap = 0 if isinstance(inst, bass_rust.InstISA) else maxw
                if si is not None and len(si.on_wait) > cap:
                    waits = list(si.on_wait)
                    keep = waits[-cap:] if cap else []
                    excess = waits[:-cap] if cap else waits
                    for w in excess:
                        nop = mybir.InstNoOp(
                            name=f"{inst.name}-wsplit{n}", engine=inst.engine
                        )
                        nop.bass_nofuse = True
                        n += 1
                        nop.sync_info = mybir.SyncInfo(
                            on_wait=[w], on_update=[]
                        )
                        new.append(nop)
                    si.on_wait = keep
                new.append(inst)
            if n:
                b.instructions = new


def _build_tile(ctx, tc, nc, mode, mdt, f32, mc, io):
    x, out = io["x"], io["out"]

    def pool(name, bufs, space="SBUF"):
        return ctx.enter_context(tc.tile_pool(name=name, bufs=bufs, space=space))

    # ---- internal DRAM: per-chunk proj-partial RS buffers ----
    dram = pool("dram", 1, space="DRAM")
    ar_c = [dram.tile([TC, E], mdt, name=f"ar{c}") for c in range(NTC)]
    rs_c = [dram.tile([TC // 2, E], mdt, name=f"rs{c}") for c in range(NTC)]

    # ---- persistent SBUF: weights & constants.  Loads needed first (LN1
    # params, identity, qkv weights) go on the sync queue; everything needed
    # later streams in parallel on the idle GpSimd DMA queue so the first
    # chunk's x loads aren't stuck behind them. ----
    wpool = pool("weights", 1)
    ln_sb = {}
    for nm in ("ln1g", "ln1b"):
        ln_sb[nm] = wpool.tile([ET, NET, 1], f32, name=nm + "_sb")
        nc.sync.dma_start(
            out=ln_sb[nm], in_=io[nm].rearrange("(k p) o -> p k o", p=ET)
        )
    id_sb = wpool.tile([TS, TS], mdt, name="id_sb")
    nc.sync.dma_start(out=id_sb, in_=io["ident"][:])
    wq_sb = wpool.tile([ET, NET, DSL], mdt, name="wq_sb")
    wk_sb = wpool.tile([ET, NET, DSL], mdt, name="wk_sb")
    wv_sb = wpool.tile([ET, NET, DSL], mdt, name="wv_sb")
    nc.gpsimd.dma_start(out=wq_sb, in_=io["wq"].rearrange("(k p) d -> p k d", p=ET))
    nc.gpsimd.dma_start(out=wk_sb, in_=io["wk"].rearrange("(k p) d -> p k d", p=ET))
    nc.gpsimd.dma_start(out=wv_sb, in_=io["wv"].rearrange("(k p) d -> p k d", p=ET))
    mask_sb = wpool.tile([TS, NSUB, TC], mdt, name="mask_sb")
    nc.gpsimd.dma_start(out=mask_sb, in_=io["masks"][:])
    wp_sb = wpool.tile([128, NDT, E], mdt, name="wp_sb")
    nc.gpsimd.dma_start(out=wp_sb, in_=io["wp"].rearrange("(k p) d -> p k d", p=128))
    ones_row = wpool.tile([1, HS], mdt, name="ones_row")
    nc.vector.memset(ones_row, 1.0)
    w1_sb = wpool.tile([ET, NET, FFN], mdt, name="w1_sb")
    nc.gpsimd.dma_start(out=w1_sb, in_=io["w1"].rearrange("(k p) d -> p k d", p=ET))
    w2_sb = wpool.tile([FFN + 1, E], mdt, name="w2_sb")
    nc.gpsimd.dma_start(out=w2_sb, in_=io["w2e"][:])
    b1_sb = wpool.tile([FFN, 1], f32, name="b1_sb")
    nc.gpsimd.dma_start(out=b1_sb, in_=io["b1"][:])
    for nm in ("ln2g", "ln2b"):
        ln_sb[nm] = wpool.tile([ET, NET, 1], f32, name=nm + "_sb")
        nc.gpsimd.dma_start(
            out=ln_sb[nm], in_=io[nm].rearrange("(k p) o -> p k o", p=ET)
        )
    eps_sb = wpool.tile([128, 1], f32, name="eps_sb")
    nc.vector.memset(eps_sb, EPS)

    # ---- persistent SBUF: per-chunk K^T, V(+ones), Q^T ----
    kv = pool("kv", 1)
    kT_c = [kv.tile([128, NDT, TC], mdt, name=f"kT{c}") for c in range(NTC)]
    vt_c = [kv.tile([128, NSUB, HPC, HS + 1], mdt, name=f"vt{c}")
            for c in range(NTC)]
    qT_c = [kv.tile([128, NDT, TC], mdt, name=f"qT{c}") for c in range(NTC)]

    # ---- working pools ----
    xt_pool = pool("xt", 4)        # [128, E] f32: x rows for LN1
    h_pool = pool("h", 5)          # [128, E] bf16: LN output rows
    mv_pool = pool("mv", 3)
    hT_pool = pool("hT", 2)        # [128, NET, TC] bf16
    pt_pool = pool("pt", 6)        # [128, 2, TC] bf16 softmax tiles
    avs_pool = pool("avs", 3)      # [HS+1, 2, TC] bf16
    dr_pool = pool("dr", 2)        # [1, 2, TC] reciprocal denominators
    attT_pool = pool("attT", 6)    # [128, TC] bf16
    stage_pool = pool("stage", 4)  # [128, E] bf16: proj partials, rs loads
    x2_pool = pool("x2", 4)        # [128, E] f32: phase-3 residual rows
    f1_pool = pool("f1", 2)
    out_pool = pool("outp", 2)
    ps_mm = pool("ps_mm", 2, space="PSUM")   # [128, 512] (1 bank each)
    ps_sc = pool("ps_sc", 2, space="PSUM")   # [128, 2, 512] (2 banks each)
    ps_av = pool("ps_av", 2, space="PSUM")   # [HS+1, 512] (1 bank each)

    def layer_norm(x_t, out_t):
        """out_t (bf16) = (x - mean) * rsqrt(var + eps).
        rsqrt is exp(-0.5*ln(var+eps)) to stay in one ScalarE table set."""
        stats = mv_pool.tile([128, 2, nc.vector.BN_STATS_DIM], f32, name="stats")
        xg = x_t.rearrange("p (s q) -> p s q", s=2)
        for s in range(2):
            nc.vector.bn_stats(out=stats[:, s, :], in_=xg[:, s, :])
        mv = mv_pool.tile([128, 2], f32, name="mv")
        nc.vector.bn_aggr(out=mv, in_=stats)
        rstd = mv_pool.tile([128, 1], f32, name="rstd")
        nc.scalar.activation(
            out=rstd, in_=mv[:, 1:2], func=AF.Ln, bias=eps_sb, scale=1.0
        )
        nc.scalar.activation(out=rstd, in_=rstd, func=AF.Exp, scale=-0.5)
        nc.vector.tensor_scalar(
            out=out_t, in0=x_t, scalar1=mv[:, 0:1], scalar2=rstd,
            op0=mybir.AluOpType.subtract, op1=mybir.AluOpType.mult,
        )
        return out_t

    def transpose_cast(h_ts, g_sb, b_sb, hT, width):
        """PE-transpose len(h_ts) subtiles of h [128, E] into hT[:, k, :]
        (bf16), batching all of them into one PSUM tile per e-tile so the
        layernorm scale/bias fold costs one DVE op per [128, width]."""
        nsub = len(h_ts)
        for k in range(NET):
            tp = ps_mm.tile([TS, nsub * TS], mdt, name="tp", tag="mm")
            for s in range(nsub):
                nc.tensor.transpose(
                    tp[:, s * TS:(s + 1) * TS],
                    h_ts[s][:, k * ET:(k + 1) * ET], id_sb,
                )
            nc.vector.tensor_scalar(
                out=hT[:, k, 0:width], in0=tp,
                scalar1=g_sb[:, k, :], scalar2=b_sb[:, k, :],
                op0=mybir.AluOpType.mult, op1=mybir.AluOpType.add,
            )

    # =====================================================================
    # Phase 1: LN1 + transpose + QKV per chunk
    # =====================================================================
    def qkv_chunk(c):
        hT = hT_pool.tile([ET, NET, TC], mdt, name="hT")
        h_ts = []
        for s in range(NSUB):
            r0 = c * TC + s * TS
            x_t = xt_pool.tile([128, E], f32, name="x_t")
            nc.sync.dma_start(out=x_t, in_=x[r0:r0 + TS, :])
            h_t = h_pool.tile([128, E], mdt, name="h_t")
            h_ts.append(layer_norm(x_t, h_t))
        transpose_cast(h_ts, ln_sb["ln1g"], ln_sb["ln1b"], hT, TC)
        for dd in range(NDT):
            for w_sb, dst in ((wq_sb, qT_c[c]), (wk_sb, kT_c[c])):
                ps = ps_mm.tile([128, TC], f32, name="ps_qk", tag="mm")
                for k in range(NET):
                    nc.tensor.matmul(
                        ps, mc(w_sb[:, k, dd * 128:(dd + 1) * 128]),
                        mc(hT[:, k, :]),
                        start=(k == 0), stop=(k == NET - 1),
                    )
                nc.vector.tensor_copy(dst[:, dd, :], ps)
        for s in range(NSUB):
            ps = ps_mm.tile([128, DSL], f32, name="ps_v", tag="mm")
            for k in range(NET):
                nc.tensor.matmul(
                    ps, mc(hT[:, k, s * TS:(s + 1) * TS]), mc(wv_sb[:, k, :]),
                    start=(k == 0), stop=(k == NET - 1),
                )
            nc.vector.tensor_copy(
                vt_c[c][:, s, :, 0:HS],
                ps.rearrange("p (h d) -> p h d", h=HPC),
            )
            nc.gpsimd.memset(vt_c[c][:, s, :, HS:HS + 1], 1.0)

    # =====================================================================
    # Phase 2: attention + proj partials + per-chunk pair ReduceScatter
    # =====================================================================
    def finish_pair(av_sb, dr, attT):
        """Broadcast the reciprocal denominators across each head's 64
        partitions with a K=1 ones-matmul and normalize into attT (bf16)."""
        for hh in range(2):
            rb = ps_mm.tile([HS, TC], f32, name="rb", tag="mm")
            nc.tensor.matmul(
                rb, mc(ones_row), mc(dr[:, hh, :]), start=True, stop=True
            )
            nc.vector.tensor_mul(
                attT[hh * HS:(hh + 1) * HS, :], av_sb[0:HS, hh, :], rb
            )

    def attention_chunk(c):
        nkt = (c + 1) * NSUB
        attTs = []
        pending = None
        for pr in range(NDT):  # head pair = d-tile
            av_ps = [ps_av.tile([HS + 1, TC], f32, name="avp") for _ in range(2)]
            prev = None
            for i in range(nkt):
                sc2 = ps_sc.tile([TS, 2, TC], f32, name="sc2")
                for hh in range(2):
                    h0 = hh * HS
                    nc.tensor.matmul(
                        sc2[:, hh, :],
                        mc(kT_c[i // NSUB][h0:h0 + HS, pr,
                                           (i % NSUB) * TS:(i % NSUB + 1) * TS]),
                        mc(qT_c[c][h0:h0 + HS, pr, :]),
                        start=True, stop=True,
                    )
                pt2 = pt_pool.tile([TS, 2, TC], mdt, name="pt2")
                nc.scalar.activation(out=pt2, in_=sc2, func=AF.Exp, scale=SCALE)
                m = i - c * NSUB
                if m >= 0:
                    # diagonal tile: zero t_k > t_q; columns beyond
                    # (m+1)*TS are fully causal already
                    w = (m + 1) * TS
                    for hh in range(2):
                        nc.vector.tensor_mul(
                            pt2[:, hh, 0:w], pt2[:, hh, 0:w], mask_sb[:, m, 0:w]
                        )
                if prev is not None:
                    pi, ppt = prev
                    for hh in range(2):
                        nc.tensor.matmul(
                            av_ps[hh],
                            mc(vt_c[pi // NSUB][:, pi % NSUB, pr * 2 + hh, :]),
                            mc(ppt[:, hh, :]),
                            start=(pi == 0), stop=False,
                        )
                if i == 2 and pending is not None:
                    finish_pair(*pending)
                    pending = None
                prev = (i, pt2)
            pi, ppt = prev
            for hh in range(2):
                nc.tensor.matmul(
                    av_ps[hh],
                    mc(vt_c[pi // NSUB][:, pi % NSUB, pr * 2 + hh, :]),
                    mc(ppt[:, hh, :]),
                    start=(pi == 0), stop=True,
                )
            av_sb = avs_pool.tile([HS + 1, 2, TC], mdt, name="av_sb")
            for hh in range(2):
                nc.vector.tensor_copy(av_sb[:, hh, :], av_ps[hh])
            # 1/den on ScalarE as exp(-ln(den)): a [1, N] DVE reciprocal
            # would serialize ~8 cycles/element on a single lane
            dr = dr_pool.tile([1, 2, TC], mdt, name="dr")
            lden = dr_pool.tile([1, 2, TC], f32, name="lden", tag="lden")
            nc.scalar.activation(
                out=lden, in_=av_sb[HS:HS + 1, :, :], func=AF.Ln
            )
            nc.scalar.activation(out=dr, in_=lden, func=AF.Exp, scale=-1.0)
            attT = attT_pool.tile([128, TC], mdt, name="attT")
            attTs.append(attT)
            if pending is not None:
                finish_pair(*pending)
            pending = (av_sb, dr, attT)
        finish_pair(*pending)
        return attTs

    def proj_chunk(c, attTs):
        for s in range(NSUB):
            part = stage_pool.tile([128, E], mdt, name="part", tag="stg")
            for n in range(2):
                ps = ps_mm.tile([128, TC], f32, name="ps_pr", tag="mm")
                for dd in range(NDT):
                    nc.tensor.matmul(
                        ps, mc(attTs[dd][:, s * TS:(s + 1) * TS]),
                        mc(wp_sb[:, dd, n * TC:(n + 1) * TC]),
                        start=(dd == 0), stop=(dd == NDT - 1),
                    )
                nc.vector.tensor_copy(part[:, n * TC:(n + 1) * TC], ps)
            nc.sync.dma_start(out=ar_c[c][s * TS:(s + 1) * TS, :], in_=part)
        nc.gpsimd.collective_compute(
            "ReduceScatter", mybir.AluOpType.add, replica_groups=PAIRS,
            ins=[ar_c[c][:]], outs=[rs_c[c]],
        )

    # =====================================================================
    # Phase 3: residual + LN2 + FFN on this core's 256-token shard of chunk c
    # =====================================================================
    def ffn_chunk(c):
        x2_ts = []
        h2_ts = []
        for s in range(2):
            rs_sb = stage_pool.tile([128, E], mdt, name="rs_sb", tag="stg")
            nc.gpsimd.dma_start(out=rs_sb, in_=rs_c[c][s * TS:(s + 1) * TS, :])
            x2_t = x2_pool.tile([128, E], f32, name="x2_t")
            nc.gpsimd.dma_start(
                out=x2_t, in_=io["x_own"][c, s * TS:(s + 1) * TS, :]
            )
            nc.vector.tensor_add(x2_t, x2_t, rs_sb)
            x2_ts.append(x2_t)
            h2_t = h_pool.tile([128, E], mdt, name="h2_t", tag="h_t")
            layer_norm(x2_t, h2_t)
            h2_ts.append(h2_t)
        h2T = hT_pool.tile([ET, NET, TC // 2], mdt, name="h2T")
        transpose_cast(h2_ts, ln_sb["ln2g"], ln_sb["ln2b"], h2T, TC // 2)
        f1 = f1_pool.tile([FFN + 1, TC // 2], mdt, name="f1")
        nc.vector.memset(f1, 1.0)  # row FFN stays 1.0 (b2 matmul row)
        ps_f = ps_mm.tile([FFN, TC // 2], f32, name="ps_f", tag="mm")
        for k in range(NET):
            nc.tensor.matmul(
                ps_f, mc(w1_sb[:, k, :]), mc(h2T[:, k, :]),
                start=(k == 0), stop=(k == NET - 1),
            )
        nc.scalar.activation(
            out=f1[0:FFN, :], in_=ps_f, func=AF.Relu, bias=b1_sb, scale=1.0
        )
        for s in range(2):
            o_t = out_pool.tile([128, E], f32, name="o_t")
            for n in range(2):
                ps = ps_mm.tile([128, TC], f32, name="ps_o", tag="mm")
                nc.tensor.matmul(
                    ps, mc(f1[:, s * TS:(s + 1) * TS]),
                    mc(w2_sb[:, n * TC:(n + 1) * TC]),
                    start=True, stop=True,
                )
                nc.vector.tensor_add(
                    o_t[:, n * TC:(n + 1) * TC], ps,
                    x2_ts[s][:, n * TC:(n + 1) * TC],
                )
            nc.sync.dma_start(out=out[c, s * TS:(s + 1) * TS, :], in_=o_t)

    # ---- schedule: qkv all chunks; attention biggest-chunk-first so the
    # big ReduceScatters get the most overlap, with each chunk's FFN emitted
    # once a later RS is in flight so only the final small pieces are exposed.
    for c in range(NTC):
        qkv_chunk(c)
    order = list(range(NTC - 1, -1, -1))  # 3, 2, 1, 0
    for idx, c in enumerate(order):
        attTs = attention_chunk(c)
        proj_chunk(c, attTs)
        if idx >= 1:
            ffn_chunk(order[idx - 1])
    ffn_chunk(order[-1])


# =========================================================================
# Host side
# =========================================================================
def _make_masks(np_mdt):
    # masks[p, d, f] = 1 iff t_k <= t_q for the diagonal block at offset d,
    # i.e. f >= 128*d + p  (t_k = 128*i + p, t_q = 512*c + f, i = 4*c + d)
    m = np.zeros((TS, NSUB, TC), dtype=np.float32)
    for d in range(NSUB):
        for p in range(TS):
            m[p, d, d * TS + p:] = 1.0
    return m.astype(np_mdt)


_NC_CACHE = {}
RUN_KWARGS = {}      # test harness may set {"trace": True} for profiling
LAST_RESULT = None   # BassKernelResults of the most recent run


def kernel(x, wq, wk, wv, w_proj, b_proj, w1, b1, w2, b2, ln1_g, ln1_b, ln2_g,
           ln2_b):
    mode = MM_MODE
    np_mdt = _np_mdt(mode)
    if mode not in _NC_CACHE:
        _NC_CACHE[mode] = build(mode)
    nc = _NC_CACHE[mode]

    x = np.asarray(x, np.float32)
    bp = np.asarray(b_proj, np.float32)
    masks = _make_masks(np_mdt)
    identity = np.eye(TS, dtype=np.float32)
    w2e = np.concatenate([np.asarray(w2, np.float32),
                          np.asarray(b2, np.float32)[None, :]], axis=0)
    in_maps = []
    for core in range(NCORE):
        b, g = core // 2, core % 2
        sl = slice(g * DSL, (g + 1) * DSL)
        # rows this core owns after the per-chunk pair ReduceScatter
        x_own = np.stack(
            [x[b, c * TC + g * (TC // 2):c * TC + (g + 1) * (TC // 2), :]
             for c in range(NTC)]
        ) + bp[None, None, :]
        in_maps.append({
            "x": x[b],
            "x_own": x_own,
            "wq": np.asarray(wq, np.float32)[:, sl].astype(np_mdt),
            "wk": np.asarray(wk, np.float32)[:, sl].astype(np_mdt),
            "wv": np.asarray(wv, np.float32)[:, sl].astype(np_mdt),
            "wp": np.asarray(w_proj, np.float32)[sl, :].astype(np_mdt),
            "w1": np.asarray(w1, np.float32).astype(np_mdt),
            "w2e": w2e.astype(np_mdt),
            "b1": np.asarray(b1, np.float32)[:, None],
            "ln1g": np.asarray(ln1_g, np.float32)[:, None],
            "ln1b": np.asarray(ln1_b, np.float32)[:, None],
            "ln2g": np.asarray(ln2_g, np.float32)[:, None],
            "ln2b": np.asarray(ln2_b, np.float32)[:, None],
            "masks": masks,
            "ident": identity.astype(np_mdt),
        })
    global LAST_RESULT
    res = run_bass_kernel_spmd(nc, in_maps, list(range(NCORE)), **RUN_KWARGS)
    LAST_RESULT = res
    outp = np.empty((B, T, E), np.float32)
    for core in range(NCORE):
        b, g = core // 2, core % 2
        o = res.results[core]["out"]
        for c in range(NTC):
            r0 = c * TC + g * (TC // 2)
            outp[b, r0:r0 + TC // 2, :] = o[c]
    return outp


# revision 23
# speedup vs baseline: 2.9154x; 1.0315x over previous
"""Trainium2 Bass kernel: pre-LN transformer block (B=4, T=2048, E=1024, H=16, FFN=100).

Sharding (8 NeuronCores): core 2b+g handles batch b, head-group g (8 of 16 heads,
i.e. a 512-wide slice of the QKV output dim / proj input dim).  Both cores of a
pair compute attention + proj partials for all 2048 tokens of their batch; a
per-chunk (512-token) pair ReduceScatter in bf16 combines the pure proj partials
and hands each core 256 tokens of the chunk, on which it runs the residual add
(x + b_proj folded host-side into the per-core x_own input), LN2 + FFN, and
writes its [4, 256, 1024] output shard.  The four chunk-RS calls are issued as
soon as each chunk's proj is done so they overlap the next chunks' attention;
per-chunk FFN work is interleaved between attention chunks the same way.

Attention layout: scores are computed transposed, S^T[t_k, t_q] = k^T.T @ q^T,
with q^T/k^T in [head_dim, token] layout (from PE-transposed LN output).  The
two heads of a d-tile pair occupy partitions 0-63 / 64-127, and their score
matmuls write the two halves of one 2-bank PSUM tile so a single ScalarE exp
(1/sqrt(E) scale folded in) covers both.  Causal masking multiplies diagonal
tiles by precomputed patterns; the softmax denominator comes from a ones column
appended to V, is reciprocated on VectorE, and is broadcast across the head's
64 partitions with a K=1 ones-matmul into PSUM (no DRAM bounce).  LayerNorm
rsqrt is computed as exp(-0.5*ln(var+eps)) so the whole kernel uses a single
ScalarE table set (natural_log_exp_and_others).
"""

from contextlib import ExitStack

import numpy as np
import ml_dtypes

import concourse.bass as bass
import concourse.mybir as mybir
import concourse.tile as tile
from concourse.bass_utils import run_bass_kernel_spmd
from concourse.vector_clock import ScopedClock


class SplitDrainTC(tile.TileContext):
    """Works around a walrus codegen limit: an SP CTRL instruction may carry
    only one sync wait, so the kernel-tail drain's waits are split onto
    preceding single-wait nops."""

    def _drain_and_barrier(self, tick_clock, wait_clock):
        probe = self.nc.sync.nop(nofuse=True)
        wait_clock.add_sem_waits(
            probe.ins, ScopedClock({None: tick_clock.global_clock})
        )
        si = probe.ins.sync_info
        waits = list(si.on_wait) if si is not None else []
        if len(waits) > 1:
            si.on_wait = [waits[0]]
            for w in waits[1:]:
                n2 = self.nc.sync.nop(nofuse=True)
                n2.ins.sync_info = mybir.SyncInfo(on_wait=[w], on_update=[])
        self.nc.sync.drain()
        self.nc.all_engine_barrier()
        popped = self.nc._tile_sem_poison_stack.pop()
        assert popped is self._sem_poison
        self.nc.clear_and_free_semaphores(list(self.sems.allocated().values()))
        self.nc.all_engine_barrier()

B, T, E, H, HS, FFN = 4, 2048, 1024, 16, 64, 100
EPS = 1e-5
NCORE = 8
TC = 512            # token chunk
NTC = T // TC       # 4
TS = 128            # token subtile
NSUB = TC // TS     # 4
ET = 128            # embed tile
NET = E // ET       # 8
DSL = E // 2        # per-core qkv output slice (8 heads * 64)
NDT = DSL // 128    # 4 d-tiles (2 heads each)
HPC = H // 2        # 8 heads per core
SCALE = float(E) ** -0.5
PAIRS = [[0, 1], [2, 3], [4, 5], [6, 7]]

MM_MODE = "bf16"    # "bf16" | "f32r" | "f32"
AF = mybir.ActivationFunctionType


def _mdt(mode):
    return mybir.dt.bfloat16 if mode == "bf16" else mybir.dt.float32


def _np_mdt(mode):
    return ml_dtypes.bfloat16 if mode == "bf16" else np.float32


def build(mode=MM_MODE):
    f32 = mybir.dt.float32
    mdt = _mdt(mode)

    def mc(ap):
        """Cast an AP for use as a matmul operand."""
        if mode == "f32r":
            return ap.bitcast(mybir.dt.float32r)
        return ap

    nc = bass.Bass(num_devices=NCORE)

    io = {}

    def param(name, shape, dtype):
        io[name] = nc.declare_dram_parameter(name, shape, dtype, isOutput=False)

    param("x", [T, E], f32)
    param("x_own", [NTC, TC // 2, E], f32)   # own scattered rows, + b_proj
    param("wq", [E, DSL], mdt)
    param("wk", [E, DSL], mdt)
    param("wv", [E, DSL], mdt)
    param("wp", [DSL, E], mdt)
    param("w1", [E, FFN], mdt)
    param("w2e", [FFN + 1, E], mdt)    # w2 with b2 as the extra last row
    param("b1", [FFN, 1], f32)
    param("ln1g", [E, 1], f32)
    param("ln1b", [E, 1], f32)
    param("ln2g", [E, 1], f32)
    param("ln2b", [E, 1], f32)
    param("masks", [TS, NSUB, TC], mdt)
    param("ident", [TS, TS], mdt)
    io["out"] = nc.declare_dram_parameter(
        "out", [NTC, TC // 2, E], f32, isOutput=True
    )

    with SplitDrainTC(nc) as tc:
        with ExitStack() as ctx:
            _build_tile(ctx, tc, nc, mode, mdt, f32, mc, io)
    _split_waits(nc)
    return nc


def _split_waits(nc, maxw=1):
    """walrus codegen accepts a limited number of sync waits per instruction;
    move the excess onto same-engine NoOps inserted just before."""
    import bass_rust
    n = 0
    for f in nc.m.functions:
        for b in f.blocks:
            new = []
            for inst in b.instructions:
                si = inst.sync_info
                # fixed-length ISA instructions can't carry waits at all
                cap = 0 if isinstance(inst, bass_rust.InstISA) else maxw
                if si is not None and len(si.on_wait) > cap:
                    waits = list(si.on_wait)
                    keep = waits[-cap:] if cap else []
                    excess = waits[:-cap] if cap else waits
                    for w in excess:
                        nop = mybir.InstNoOp(
                            name=f"{inst.name}-wsplit{n}", engine=inst.engine
                        )
                        nop.bass_nofuse = True
                        n += 1
                        nop.sync_info = mybir.SyncInfo(
                            on_wait=[w], on_update=[]
                        )
                        new.append(nop)
                    si.on_wait = keep
                new.append(inst)
            if n:
                b.instructions = new


def _build_tile(ctx, tc, nc, mode, mdt, f32, mc, io):
    x, out = io["x"], io["out"]

    def pool(name, bufs, space="SBUF"):
        return ctx.enter_context(tc.tile_pool(name=name, bufs=bufs, space=space))

    # ---- internal DRAM: per-chunk proj-partial RS buffers ----
    dram = pool("dram", 1, space="DRAM")
    ar_c = [dram.tile([TC, E], mdt, name=f"ar{c}") for c in range(NTC)]
    rs_c = [dram.tile([TC // 2, E], mdt, name=f"rs{c}") for c in range(NTC)]

    # ---- persistent SBUF: weights & constants.  DMA dependencies are
    # tracked by a shared ordinal counter, so anything emitted before the
    # first chunk's x loads delays the first LayerNorm: only LN1 params,
    # identity and qkv weights are emitted up front; the rest is emitted
    # after chunk 0's x loads (see load_late_weights below). ----
    wpool = pool("weights", 1)
    ln_sb = {}
    for nm in ("ln1g", "ln1b"):
        ln_sb[nm] = wpool.tile([ET, NET, 1], f32, name=nm + "_sb")
        nc.sync.dma_start(
            out=ln_sb[nm], in_=io[nm].rearrange("(k p) o -> p k o", p=ET)
        )
    id_sb = wpool.tile([TS, TS], mdt, name="id_sb")
    nc.sync.dma_start(out=id_sb, in_=io["ident"][:])
    wq_sb = wpool.tile([ET, NET, DSL], mdt, name="wq_sb")
    wk_sb = wpool.tile([ET, NET, DSL], mdt, name="wk_sb")
    wv_sb = wpool.tile([ET, NET, DSL], mdt, name="wv_sb")
    nc.gpsimd.dma_start(out=wq_sb, in_=io["wq"].rearrange("(k p) d -> p k d", p=ET))
    nc.gpsimd.dma_start(out=wk_sb, in_=io["wk"].rearrange("(k p) d -> p k d", p=ET))
    nc.gpsimd.dma_start(out=wv_sb, in_=io["wv"].rearrange("(k p) d -> p k d", p=ET))
    ones_row = wpool.tile([1, HS], mdt, name="ones_row")
    nc.vector.memset(ones_row, 1.0)
    eps_sb = wpool.tile([128, 1], f32, name="eps_sb")
    nc.vector.memset(eps_sb, EPS)
    mask_sb = wpool.tile([TS, NSUB, TC], mdt, name="mask_sb")
    wp_sb = wpool.tile([128, NDT, E], mdt, name="wp_sb")
    w1_sb = wpool.tile([ET, NET, FFN], mdt, name="w1_sb")
    w2_sb = wpool.tile([FFN + 1, E], mdt, name="w2_sb")
    b1_sb = wpool.tile([FFN, 1], f32, name="b1_sb")

    def load_late_weights():
        nc.gpsimd.dma_start(out=mask_sb, in_=io["masks"][:])
        nc.gpsimd.dma_start(
            out=wp_sb, in_=io["wp"].rearrange("(k p) d -> p k d", p=128)
        )
        nc.gpsimd.dma_start(
            out=w1_sb, in_=io["w1"].rearrange("(k p) d -> p k d", p=ET)
        )
        nc.gpsimd.dma_start(out=w2_sb, in_=io["w2e"][:])
        nc.gpsimd.dma_start(out=b1_sb, in_=io["b1"][:])
        for nm in ("ln2g", "ln2b"):
            ln_sb[nm] = wpool.tile([ET, NET, 1], f32, name=nm + "_sb")
            nc.gpsimd.dma_start(
                out=ln_sb[nm], in_=io[nm].rearrange("(k p) o -> p k o", p=ET)
            )

    # ---- persistent SBUF: per-chunk K^T, V(+ones), Q^T ----
    kv = pool("kv", 1)
    kT_c = [kv.tile([128, NDT, TC], mdt, name=f"kT{c}") for c in range(NTC)]
    vt_c = [kv.tile([128, NSUB, HPC, HS + 1], mdt, name=f"vt{c}")
            for c in range(NTC)]
    qT_c = [kv.tile([128, NDT, TC], mdt, name=f"qT{c}") for c in range(NTC)]

    # ---- working pools ----
    xt_pool = pool("xt", 4)        # [128, E] f32: x rows for LN1
    h_pool = pool("h", 5)          # [128, E] bf16: LN output rows
    mv_pool = pool("mv", 3)
    hT_pool = pool("hT", 2)        # [128, NET, TC] bf16
    pt_pool = pool("pt", 6)        # [128, 2, TC] bf16 softmax tiles
    avs_pool = pool("avs", 3)      # [HS+1, 2, TC] bf16
    dr_pool = pool("dr", 2)        # [1, 2, TC] reciprocal denominators
    attT_pool = pool("attT", 6)    # [128, TC] bf16
    stage_pool = pool("stage", 4)  # [128, E] bf16: proj partials, rs loads
    x2_pool = pool("x2", 4)        # [128, E] f32: phase-3 residual rows
    f1_pool = pool("f1", 2)
    out_pool = pool("outp", 2)
    ps_mm = pool("ps_mm", 2, space="PSUM")   # [128, 512] (1 bank each)
    ps_sc = pool("ps_sc", 2, space="PSUM")   # [128, 2, 512] (2 banks each)
    ps_av = pool("ps_av", 2, space="PSUM")   # [HS+1, 512] (1 bank each)

    def layer_norm(x_t, out_t):
        """out_t (bf16) = (x - mean) * rsqrt(var + eps).
        rsqrt is exp(-0.5*ln(var+eps)) to stay in one ScalarE table set."""
        stats = mv_pool.tile([128, 2, nc.vector.BN_STATS_DIM], f32, name="stats")
        xg = x_t.rearrange("p (s q) -> p s q", s=2)
        for s in range(2):
            nc.vector.bn_stats(out=stats[:, s, :], in_=xg[:, s, :])
        mv = mv_pool.tile([128, 2], f32, name="mv")
        nc.vector.bn_aggr(out=mv, in_=stats)
        rstd = mv_pool.tile([128, 1], f32, name="rstd")
        nc.scalar.activation(
            out=rstd, in_=mv[:, 1:2], func=AF.Ln, bias=eps_sb, scale=1.0
        )
        nc.scalar.activation(out=rstd, in_=rstd, func=AF.Exp, scale=-0.5)
        nc.vector.tensor_scalar(
            out=out_t, in0=x_t, scalar1=mv[:, 0:1], scalar2=rstd,
            op0=mybir.AluOpType.subtract, op1=mybir.AluOpType.mult,
        )
        return out_t

    def transpose_cast(h_ts, g_sb, b_sb, hT, width):
        """PE-transpose len(h_ts) subtiles of h [128, E] into hT[:, k, :]
        (bf16), batching all of them into one PSUM tile per e-tile so the
        layernorm scale/bias fold costs one DVE op per [128, width]."""
        nsub = len(h_ts)
        for k in range(NET):
            tp = ps_mm.tile([TS, nsub * TS], mdt, name="tp", tag="mm")
            for s in range(nsub):
                nc.tensor.transpose(
                    tp[:, s * TS:(s + 1) * TS],
                    h_ts[s][:, k * ET:(k + 1) * ET], id_sb,
                )
            nc.vector.tensor_scalar(
                out=hT[:, k, 0:width], in0=tp,
                scalar1=g_sb[:, k, :], scalar2=b_sb[:, k, :],
                op0=mybir.AluOpType.mult, op1=mybir.AluOpType.add,
            )

    # =====================================================================
    # Phase 1: LN1 + transpose + QKV per chunk
    # =====================================================================
    def ln1_chunk(c):
        h_ts = []
        for s in range(NSUB):
            r0 = c * TC + s * TS
            x_t = xt_pool.tile([128, E], f32, name="x_t")
            nc.sync.dma_start(out=x_t, in_=x[r0:r0 + TS, :])
            h_t = h_pool.tile([128, E], mdt, name="h_t")
            h_ts.append(layer_norm(x_t, h_t))
        return h_ts

    def qkv_chunk(c, h_ts):
        hT = hT_pool.tile([ET, NET, TC], mdt, name="hT")
        transpose_cast(h_ts, ln_sb["ln1g"], ln_sb["ln1b"], hT, TC)
        for dd in range(NDT):
            for w_sb, dst in ((wq_sb, qT_c[c]), (wk_sb, kT_c[c])):
                ps = ps_mm.tile([128, TC], f32, name="ps_qk", tag="mm")
                for k in range(NET):
                    nc.tensor.matmul(
                        ps, mc(w_sb[:, k, dd * 128:(dd + 1) * 128]),
                        mc(hT[:, k, :]),
                        start=(k == 0), stop=(k == NET - 1),
                    )
                nc.vector.tensor_copy(dst[:, dd, :], ps)
        for s in range(NSUB):
            ps = ps_mm.tile([128, DSL], f32, name="ps_v", tag="mm")
            for k in range(NET):
                nc.tensor.matmul(
                    ps, mc(hT[:, k, s * TS:(s + 1) * TS]), mc(wv_sb[:, k, :]),
                    start=(k == 0), stop=(k == NET - 1),
                )
            nc.vector.tensor_copy(
                vt_c[c][:, s, :, 0:HS],
                ps.rearrange("p (h d) -> p h d", h=HPC),
            )
            nc.gpsimd.memset(vt_c[c][:, s, :, HS:HS + 1], 1.0)

    # =====================================================================
    # Phase 2: attention + proj partials + per-chunk pair ReduceScatter
    # =====================================================================
    def finish_pair(av_sb, dr, attT):
        """Broadcast the reciprocal denominators across each head's 64
        partitions with a K=1 ones-matmul and normalize into attT (bf16)."""
        for hh in range(2):
            rb = ps_mm.tile([HS, TC], f32, name="rb", tag="mm")
            nc.tensor.matmul(
                rb, mc(ones_row), mc(dr[:, hh, :]), start=True, stop=True
            )
            nc.vector.tensor_mul(
                attT[hh * HS:(hh + 1) * HS, :], av_sb[0:HS, hh, :], rb
            )

    def attention_chunk(c):
        nkt = (c + 1) * NSUB
        attTs = []
        pending = None
        for pr in range(NDT):  # head pair = d-tile
            av_ps = [ps_av.tile([HS + 1, TC], f32, name="avp") for _ in range(2)]
            def av_mms(pi, ppt, p0, last):
                # columns below p0 get no contribution from this t_k tile
                # (fully above the diagonal); PSUM accumulation is
                # per-element so the shorter matmul leaves them untouched
                for hh in range(2):
                    nc.tensor.matmul(
                        av_ps[hh][:, p0:TC],
                        mc(vt_c[pi // NSUB][:, pi % NSUB, pr * 2 + hh, :]),
                        mc(ppt[:, hh, p0:TC]),
                        start=(pi == 0), stop=last,
                    )

            prev = None
            for i in range(nkt):
                m = i - c * NSUB
                # p0: first t_q column this t_k tile can attend to
                p0 = m * TS if m > 0 else 0
                sc2 = ps_sc.tile([TS, 2, TC], f32, name="sc2")
                for hh in range(2):
                    h0 = hh * HS
                    nc.tensor.matmul(
                        sc2[:, hh, p0:TC],
                        mc(kT_c[i // NSUB][h0:h0 + HS, pr,
                                           (i % NSUB) * TS:(i % NSUB + 1) * TS]),
                        mc(qT_c[c][h0:h0 + HS, pr, p0:TC]),
                        start=True, stop=True,
                    )
                pt2 = pt_pool.tile([TS, 2, TC], mdt, name="pt2")
                nc.scalar.activation(
                    out=pt2[:, :, p0:TC], in_=sc2[:, :, p0:TC],
                    func=AF.Exp, scale=SCALE,
                )
                if m >= 0:
                    # diagonal TS block: zero t_k > t_q within it
                    for hh in range(2):
                        nc.vector.tensor_mul(
                            pt2[:, hh, p0:p0 + TS], pt2[:, hh, p0:p0 + TS],
                            mask_sb[:, m, p0:p0 + TS],
                        )
                if prev is not None:
                    av_mms(*prev, last=False)
                if i == 2 and pending is not None:
                    finish_pair(*pending)
                    pending = None
                prev = (i, pt2, p0)
            av_mms(*prev, last=True)
            av_sb = avs_pool.tile([HS + 1, 2, TC], mdt, name="av_sb")
            for hh in range(2):
                nc.vector.tensor_copy(av_sb[:, hh, :], av_ps[hh])
            # 1/den on ScalarE as exp(-ln(den)): a [1, N] DVE reciprocal
            # would serialize ~8 cycles/element on a single lane
            dr = dr_pool.tile([1, 2, TC], mdt, name="dr")
            lden = dr_pool.tile([1, 2, TC], f32, name="lden", tag="lden")
            nc.scalar.activation(
                out=lden, in_=av_sb[HS:HS + 1, :, :], func=AF.Ln
            )
            nc.scalar.activation(out=dr, in_=lden, func=AF.Exp, scale=-1.0)
            attT = attT_pool.tile([128, TC], mdt, name="attT")
            attTs.append(attT)
            if pending is not None:
                finish_pair(*pending)
            pending = (av_sb, dr, attT)
        finish_pair(*pending)
        return attTs

    def proj_chunk(c, attTs):
        for s in range(NSUB):
            part = stage_pool.tile([128, E], mdt, name="part", tag="stg")
            for n in range(2):
                ps = ps_mm.tile([128, TC], f32, name="ps_pr", tag="mm")
                for dd in range(NDT):
                    nc.tensor.matmul(
                        ps, mc(attTs[dd][:, s * TS:(s + 1) * TS]),
                        mc(wp_sb[:, dd, n * TC:(n + 1) * TC]),
                        start=(dd == 0), stop=(dd == NDT - 1),
                    )
                nc.vector.tensor_copy(part[:, n * TC:(n + 1) * TC], ps)
            nc.sync.dma_start(out=ar_c[c][s * TS:(s + 1) * TS, :], in_=part)
        nc.gpsimd.collective_compute(
            "ReduceScatter", mybir.AluOpType.add, replica_groups=PAIRS,
            ins=[ar_c[c][:]], outs=[rs_c[c]],
        )

    # =====================================================================
    # Phase 3: residual + LN2 + FFN on this core's 256-token shard of chunk c
    # =====================================================================
    def ffn_chunk(c):
        x2_ts = []
        h2_ts = []
        for s in range(2):
            rs_sb = stage_pool.tile([128, E], mdt, name="rs_sb", tag="stg")
            nc.sync.dma_start(out=rs_sb, in_=rs_c[c][s * TS:(s + 1) * TS, :])
            x2_t = x2_pool.tile([128, E], f32, name="x2_t")
            nc.sync.dma_start(
                out=x2_t, in_=io["x_own"][c, s * TS:(s + 1) * TS, :]
            )
            nc.vector.tensor_add(x2_t, x2_t, rs_sb)
            x2_ts.append(x2_t)
            h2_t = h_pool.tile([128, E], mdt, name="h2_t", tag="h_t")
            layer_norm(x2_t, h2_t)
            h2_ts.append(h2_t)
        h2T = hT_pool.tile([ET, NET, TC // 2], mdt, name="h2T")
        transpose_cast(h2_ts, ln_sb["ln2g"], ln_sb["ln2b"], h2T, TC // 2)
        f1 = f1_pool.tile([FFN + 1, TC // 2], mdt, name="f1")
        nc.vector.memset(f1, 1.0)  # row FFN stays 1.0 (b2 matmul row)
        ps_f = ps_mm.tile([FFN, TC // 2], f32, name="ps_f", tag="mm")
        for k in range(NET):
            nc.tensor.matmul(
                ps_f, mc(w1_sb[:, k, :]), mc(h2T[:, k, :]),
                start=(k == 0), stop=(k == NET - 1),
            )
        nc.scalar.activation(
            out=f1[0:FFN, :], in_=ps_f, func=AF.Relu, bias=b1_sb, scale=1.0
        )
        for s in range(2):
            o_t = out_pool.tile([128, E], f32, name="o_t")
            for n in range(2):
                ps = ps_mm.tile([128, TC], f32, name="ps_o", tag="mm")
                nc.tensor.matmul(
                    ps, mc(f1[:, s * TS:(s + 1) * TS]),
                    mc(w2_sb[:, n * TC:(n + 1) * TC]),
                    start=True, stop=True,
                )
                nc.vector.tensor_add(
                    o_t[:, n * TC:(n + 1) * TC], ps,
                    x2_ts[s][:, n * TC:(n + 1) * TC],
                )
            nc.sync.dma_start(out=out[c, s * TS:(s + 1) * TS, :], in_=o_t)

    # ---- schedule: qkv all chunks; attention biggest-chunk-first so the
    # big ReduceScatters get the most overlap, with each chunk's FFN emitted
    # once a later RS is in flight so only the final small pieces are exposed.
    h0 = ln1_chunk(0)
    load_late_weights()  # after chunk 0's x loads in DMA-ordinal order
    qkv_chunk(0, h0)
    for c in range(1, NTC):
        qkv_chunk(c, ln1_chunk(c))
    order = list(range(NTC - 1, -1, -1))  # 3, 2, 1, 0
    for idx, c in enumerate(order):
        attTs = attention_chunk(c)
        proj_chunk(c, attTs)
        if idx >= 1:
            ffn_chunk(order[idx - 1])
    ffn_chunk(order[-1])


# =========================================================================
# Host side
# =========================================================================
def _make_masks(np_mdt):
    # masks[p, d, f] = 1 iff t_k <= t_q for the diagonal block at offset d,
    # i.e. f >= 128*d + p  (t_k = 128*i + p, t_q = 512*c + f, i = 4*c + d)
    m = np.zeros((TS, NSUB, TC), dtype=np.float32)
    for d in range(NSUB):
        for p in range(TS):
            m[p, d, d * TS + p:] = 1.0
    return m.astype(np_mdt)


_NC_CACHE = {}
RUN_KWARGS = {}      # test harness may set {"trace": True} for profiling
LAST_RESULT = None   # BassKernelResults of the most recent run


def kernel(x, wq, wk, wv, w_proj, b_proj, w1, b1, w2, b2, ln1_g, ln1_b, ln2_g,
           ln2_b):
    mode = MM_MODE
    np_mdt = _np_mdt(mode)
    if mode not in _NC_CACHE:
        _NC_CACHE[mode] = build(mode)
    nc = _NC_CACHE[mode]

    x = np.asarray(x, np.float32)
    bp = np.asarray(b_proj, np.float32)
    masks = _make_masks(np_mdt)
    identity = np.eye(TS, dtype=np.float32)
    w2e = np.concatenate([np.asarray(w2, np.float32),
                          np.asarray(b2, np.float32)[None, :]], axis=0)
    in_maps = []
    for core in range(NCORE):
        b, g = core // 2, core % 2
        sl = slice(g * DSL, (g + 1) * DSL)
        # rows this core owns after the per-chunk pair ReduceScatter
        x_own = np.stack(
            [x[b, c * TC + g * (TC // 2):c * TC + (g + 1) * (TC // 2), :]
             for c in range(NTC)]
        ) + bp[None, None, :]
        in_maps.append({
            "x": x[b],
            "x_own": x_own,
            "wq": np.asarray(wq, np.float32)[:, sl].astype(np_mdt),
            "wk": np.asarray(wk, np.float32)[:, sl].astype(np_mdt),
            "wv": np.asarray(wv, np.float32)[:, sl].astype(np_mdt),
            "wp": np.asarray(w_proj, np.float32)[sl, :].astype(np_mdt),
            "w1": np.asarray(w1, np.float32).astype(np_mdt),
            "w2e": w2e.astype(np_mdt),
            "b1": np.asarray(b1, np.float32)[:, None],
            "ln1g": np.asarray(ln1_g, np.float32)[:, None],
            "ln1b": np.asarray(ln1_b, np.float32)[:, None],
            "ln2g": np.asarray(ln2_g, np.float32)[:, None],
            "ln2b": np.asarray(ln2_b, np.float32)[:, None],
            "masks": masks,
            "ident": identity.astype(np_mdt),
        })
    global LAST_RESULT
    res = run_bass_kernel_spmd(nc, in_maps, list(range(NCORE)), **RUN_KWARGS)
    LAST_RESULT = res
    outp = np.empty((B, T, E), np.float32)
    for core in range(NCORE):
        b, g = core // 2, core % 2
        o = res.results[core]["out"]
        for c in range(NTC):
            r0 = c * TC + g * (TC // 2)
            outp[b, r0:r0 + TC // 2, :] = o[c]
    return outp


# revision 25
# speedup vs baseline: 2.9382x; 1.0078x over previous
"""Trainium2 Bass kernel: pre-LN transformer block (B=4, T=2048, E=1024, H=16, FFN=100).

Sharding (8 NeuronCores): core 2b+g handles batch b, head-group g (8 of 16 heads,
i.e. a 512-wide slice of the QKV output dim / proj input dim).  Both cores of a
pair compute attention + proj partials for all 2048 tokens of their batch; a
per-chunk (512-token) pair ReduceScatter in bf16 combines the pure proj partials
and hands each core 256 tokens of the chunk, on which it runs the residual add
(x + b_proj folded host-side into the per-core x_own input), LN2 + FFN, and
writes its [4, 256, 1024] output shard.  The four chunk-RS calls are issued as
soon as each chunk's proj is done so they overlap the next chunks' attention;
per-chunk FFN work is interleaved between attention chunks the same way.

Attention layout: scores are computed transposed, S^T[t_k, t_q] = k^T.T @ q^T,
with q^T/k^T in [head_dim, token] layout (from PE-transposed LN output).  The
two heads of a d-tile pair occupy partitions 0-63 / 64-127, and their score
matmuls write the two halves of one 2-bank PSUM tile so a single ScalarE exp
(1/sqrt(E) scale folded in) covers both.  Causal masking multiplies diagonal
tiles by precomputed patterns; the softmax denominator comes from a ones column
appended to V, is reciprocated on VectorE, and is broadcast across the head's
64 partitions with a K=1 ones-matmul into PSUM (no DRAM bounce).  LayerNorm
rsqrt is computed as exp(-0.5*ln(var+eps)) so the whole kernel uses a single
ScalarE table set (natural_log_exp_and_others).
"""

from contextlib import ExitStack

import numpy as np
import ml_dtypes

import concourse.bass as bass
import concourse.mybir as mybir
import concourse.tile as tile
from concourse.bass_utils import run_bass_kernel_spmd
from concourse.vector_clock import ScopedClock


class SplitDrainTC(tile.TileContext):
    """Works around a walrus codegen limit: an SP CTRL instruction may carry
    only one sync wait, so the kernel-tail drain's waits are split onto
    preceding single-wait nops."""

    def _drain_and_barrier(self, tick_clock, wait_clock):
        probe = self.nc.sync.nop(nofuse=True)
        wait_clock.add_sem_waits(
            probe.ins, ScopedClock({None: tick_clock.global_clock})
        )
        si = probe.ins.sync_info
        waits = list(si.on_wait) if si is not None else []
        if len(waits) > 1:
            si.on_wait = [waits[0]]
            for w in waits[1:]:
                n2 = self.nc.sync.nop(nofuse=True)
                n2.ins.sync_info = mybir.SyncInfo(on_wait=[w], on_update=[])
        self.nc.sync.drain()
        self.nc.all_engine_barrier()
        popped = self.nc._tile_sem_poison_stack.pop()
        assert popped is self._sem_poison
        self.nc.clear_and_free_semaphores(list(self.sems.allocated().values()))
        self.nc.all_engine_barrier()

B, T, E, H, HS, FFN = 4, 2048, 1024, 16, 64, 100
EPS = 1e-5
NCORE = 8
TC = 512            # token chunk
NTC = T // TC       # 4
TS = 128            # token subtile
NSUB = TC // TS     # 4
ET = 128            # embed tile
NET = E // ET       # 8
DSL = E // 2        # per-core qkv output slice (8 heads * 64)
NDT = DSL // 128    # 4 d-tiles (2 heads each)
HPC = H // 2        # 8 heads per core
SCALE = float(E) ** -0.5
PAIRS = [[0, 1], [2, 3], [4, 5], [6, 7]]

MM_MODE = "bf16"    # "bf16" | "f32r" | "f32"
AF = mybir.ActivationFunctionType


def _mdt(mode):
    return mybir.dt.bfloat16 if mode == "bf16" else mybir.dt.float32


def _np_mdt(mode):
    return ml_dtypes.bfloat16 if mode == "bf16" else np.float32


def build(mode=MM_MODE):
    f32 = mybir.dt.float32
    mdt = _mdt(mode)

    def mc(ap):
        """Cast an AP for use as a matmul operand."""
        if mode == "f32r":
            return ap.bitcast(mybir.dt.float32r)
        return ap

    nc = bass.Bass(num_devices=NCORE)

    io = {}

    def param(name, shape, dtype):
        io[name] = nc.declare_dram_parameter(name, shape, dtype, isOutput=False)

    param("x", [T, E], f32)
    param("x_own", [NTC, TC // 2, E], f32)   # own scattered rows, + b_proj
    param("wq", [E, DSL], mdt)
    param("wk", [E, DSL], mdt)
    param("wv", [E, DSL], mdt)
    param("wp", [DSL, E], mdt)
    param("w1", [E, FFN], mdt)
    param("w2e", [FFN + 1, E], mdt)    # w2 with b2 as the extra last row
    param("b1", [FFN, 1], f32)
    param("ln1g", [E, 1], f32)
    param("ln1b", [E, 1], f32)
    param("ln2g", [E, 1], f32)
    param("ln2b", [E, 1], f32)
    param("masks", [TS, NSUB, TC], mdt)
    param("ident", [TS, TS], mdt)
    io["out"] = nc.declare_dram_parameter(
        "out", [NTC, TC // 2, E], f32, isOutput=True
    )

    with SplitDrainTC(nc) as tc:
        with ExitStack() as ctx:
            _build_tile(ctx, tc, nc, mode, mdt, f32, mc, io)
    _split_waits(nc)
    return nc


def _split_waits(nc, maxw=1):
    """walrus codegen accepts a limited number of sync waits per instruction;
    move the excess onto same-engine NoOps inserted just before."""
    import bass_rust
    n = 0
    for f in nc.m.functions:
        for b in f.blocks:
            new = []
            for inst in b.instructions:
                si = inst.sync_info
                # fixed-length ISA instructions can't carry waits at all
                cap = 0 if isinstance(inst, bass_rust.InstISA) else maxw
                if si is not None and len(si.on_wait) > cap:
                    waits = list(si.on_wait)
                    keep = waits[-cap:] if cap else []
                    excess = waits[:-cap] if cap else waits
                    for w in excess:
                        nop = mybir.InstNoOp(
                            name=f"{inst.name}-wsplit{n}", engine=inst.engine
                        )
                        nop.bass_nofuse = True
                        n += 1
                        nop.sync_info = mybir.SyncInfo(
                            on_wait=[w], on_update=[]
                        )
                        new.append(nop)
                    si.on_wait = keep
                new.append(inst)
            if n:
                b.instructions = new


def _build_tile(ctx, tc, nc, mode, mdt, f32, mc, io):
    x, out = io["x"], io["out"]

    def pool(name, bufs, space="SBUF"):
        return ctx.enter_context(tc.tile_pool(name=name, bufs=bufs, space=space))

    # ---- internal DRAM: per-chunk proj-partial RS buffers ----
    dram = pool("dram", 1, space="DRAM")
    ar_c = [dram.tile([TC, E], mdt, name=f"ar{c}") for c in range(NTC)]
    rs_c = [dram.tile([TC // 2, E], mdt, name=f"rs{c}") for c in range(NTC)]

    # ---- persistent SBUF: weights & constants.  DMA dependencies are
    # tracked by a shared ordinal counter, so anything emitted before the
    # first chunk's x loads delays the first LayerNorm: only LN1 params,
    # identity and qkv weights are emitted up front; the rest is emitted
    # after chunk 0's x loads (see load_late_weights below). ----
    wpool = pool("weights", 1)
    ln_sb = {}
    for nm in ("ln1g", "ln1b"):
        ln_sb[nm] = wpool.tile([ET, NET, 1], f32, name=nm + "_sb")
        nc.sync.dma_start(
            out=ln_sb[nm], in_=io[nm].rearrange("(k p) o -> p k o", p=ET)
        )
    id_sb = wpool.tile([TS, TS], mdt, name="id_sb")
    nc.gpsimd.dma_start(out=id_sb, in_=io["ident"][:])
    wq_sb = wpool.tile([ET, NET, DSL], mdt, name="wq_sb")
    wk_sb = wpool.tile([ET, NET, DSL], mdt, name="wk_sb")
    wv_sb = wpool.tile([ET, NET, DSL], mdt, name="wv_sb")
    nc.gpsimd.dma_start(out=wq_sb, in_=io["wq"].rearrange("(k p) d -> p k d", p=ET))
    nc.gpsimd.dma_start(out=wk_sb, in_=io["wk"].rearrange("(k p) d -> p k d", p=ET))
    nc.gpsimd.dma_start(out=wv_sb, in_=io["wv"].rearrange("(k p) d -> p k d", p=ET))
    ones_row = wpool.tile([1, HS], mdt, name="ones_row")
    nc.vector.memset(ones_row, 1.0)
    eps_sb = wpool.tile([128, 1], f32, name="eps_sb")
    nc.vector.memset(eps_sb, EPS)
    mask_sb = wpool.tile([TS, NSUB, TC], mdt, name="mask_sb")
    wp_sb = wpool.tile([128, NDT, E], mdt, name="wp_sb")
    w1_sb = wpool.tile([ET, NET, FFN], mdt, name="w1_sb")
    w2_sb = wpool.tile([FFN + 1, E], mdt, name="w2_sb")
    b1_sb = wpool.tile([FFN, 1], f32, name="b1_sb")

    def load_late_weights():
        nc.gpsimd.dma_start(out=mask_sb, in_=io["masks"][:])
        nc.gpsimd.dma_start(
            out=wp_sb, in_=io["wp"].rearrange("(k p) d -> p k d", p=128)
        )
        nc.gpsimd.dma_start(
            out=w1_sb, in_=io["w1"].rearrange("(k p) d -> p k d", p=ET)
        )
        nc.gpsimd.dma_start(out=w2_sb, in_=io["w2e"][:])
        nc.gpsimd.dma_start(out=b1_sb, in_=io["b1"][:])
        for nm in ("ln2g", "ln2b"):
            ln_sb[nm] = wpool.tile([ET, NET, 1], f32, name=nm + "_sb")
            nc.gpsimd.dma_start(
                out=ln_sb[nm], in_=io[nm].rearrange("(k p) o -> p k o", p=ET)
            )

    # ---- persistent SBUF: per-chunk K^T, V(+ones), Q^T ----
    kv = pool("kv", 1)
    kT_c = [kv.tile([128, NDT, TC], mdt, name=f"kT{c}") for c in range(NTC)]
    vt_c = [kv.tile([128, NSUB, HPC, HS + 1], mdt, name=f"vt{c}")
            for c in range(NTC)]
    qT_c = [kv.tile([128, NDT, TC], mdt, name=f"qT{c}") for c in range(NTC)]

    # ---- working pools ----
    xt_pool = pool("xt", 4)        # [128, E] f32: x rows for LN1
    h_pool = pool("h", 5)          # [128, E] bf16: LN output rows
    mv_pool = pool("mv", 3)
    hT_pool = pool("hT", 2)        # [128, NET, TC] bf16
    pt_pool = pool("pt", 6)        # [128, 2, TC] bf16 softmax tiles
    avs_pool = pool("avs", 3)      # [HS+1, 2, TC] bf16
    dr_pool = pool("dr", 2)        # [1, 2, TC] reciprocal denominators
    attT_pool = pool("attT", 6)    # [128, TC] bf16
    stage_pool = pool("stage", 4)  # [128, E] bf16: proj partials, rs loads
    x2_pool = pool("x2", 4)        # [128, E] f32: phase-3 residual rows
    f1_pool = pool("f1", 2)
    out_pool = pool("outp", 2)
    ps_mm = pool("ps_mm", 2, space="PSUM")   # [128, 512] (1 bank each)
    ps_sc = pool("ps_sc", 2, space="PSUM")   # [128, 2, 512] (2 banks each)
    ps_av = pool("ps_av", 2, space="PSUM")   # [HS+1, 512] (1 bank each)

    def layer_norm(x_t, out_t):
        """out_t (bf16) = (x - mean) * rsqrt(var + eps).
        rsqrt is exp(-0.5*ln(var+eps)) to stay in one ScalarE table set."""
        stats = mv_pool.tile([128, 2, nc.vector.BN_STATS_DIM], f32, name="stats")
        xg = x_t.rearrange("p (s q) -> p s q", s=2)
        for s in range(2):
            nc.vector.bn_stats(out=stats[:, s, :], in_=xg[:, s, :])
        mv = mv_pool.tile([128, 2], f32, name="mv")
        nc.vector.bn_aggr(out=mv, in_=stats)
        rstd = mv_pool.tile([128, 1], f32, name="rstd")
        nc.scalar.activation(
            out=rstd, in_=mv[:, 1:2], func=AF.Ln, bias=eps_sb, scale=1.0
        )
        nc.scalar.activation(out=rstd, in_=rstd, func=AF.Exp, scale=-0.5)
        nc.vector.tensor_scalar(
            out=out_t, in0=x_t, scalar1=mv[:, 0:1], scalar2=rstd,
            op0=mybir.AluOpType.subtract, op1=mybir.AluOpType.mult,
        )
        return out_t

    def transpose_cast(h_ts, g_sb, b_sb, hT, width):
        """PE-transpose len(h_ts) subtiles of h [128, E] into hT[:, k, :]
        (bf16), batching all of them into one PSUM tile per e-tile so the
        layernorm scale/bias fold costs one DVE op per [128, width]."""
        nsub = len(h_ts)
        for k in range(NET):
            tp = ps_mm.tile([TS, nsub * TS], mdt, name="tp", tag="mm")
            for s in range(nsub):
                nc.tensor.transpose(
                    tp[:, s * TS:(s + 1) * TS],
                    h_ts[s][:, k * ET:(k + 1) * ET], id_sb,
                )
            nc.vector.tensor_scalar(
                out=hT[:, k, 0:width], in0=tp,
                scalar1=g_sb[:, k, :], scalar2=b_sb[:, k, :],
                op0=mybir.AluOpType.mult, op1=mybir.AluOpType.add,
            )

    # =====================================================================
    # Phase 1: LN1 + transpose + QKV per chunk
    # =====================================================================
    def ln1_chunk(c):
        h_ts = []
        for s in range(NSUB):
            r0 = c * TC + s * TS
            x_t = xt_pool.tile([128, E], f32, name="x_t")
            nc.sync.dma_start(out=x_t, in_=x[r0:r0 + TS, :])
            h_t = h_pool.tile([128, E], mdt, name="h_t")
            h_ts.append(layer_norm(x_t, h_t))
        return h_ts

    def qkv_chunk(c, h_ts):
        hT = hT_pool.tile([ET, NET, TC], mdt, name="hT")
        transpose_cast(h_ts, ln_sb["ln1g"], ln_sb["ln1b"], hT, TC)
        for dd in range(NDT):
            for w_sb, dst in ((wq_sb, qT_c[c]), (wk_sb, kT_c[c])):
                ps = ps_mm.tile([128, TC], f32, name="ps_qk", tag="mm")
                for k in range(NET):
                    nc.tensor.matmul(
                        ps, mc(w_sb[:, k, dd * 128:(dd + 1) * 128]),
                        mc(hT[:, k, :]),
                        start=(k == 0), stop=(k == NET - 1),
                    )
                nc.vector.tensor_copy(dst[:, dd, :], ps)
        for s in range(NSUB):
            ps = ps_mm.tile([128, DSL], f32, name="ps_v", tag="mm")
            for k in range(NET):
                nc.tensor.matmul(
                    ps, mc(hT[:, k, s * TS:(s + 1) * TS]), mc(wv_sb[:, k, :]),
                    start=(k == 0), stop=(k == NET - 1),
                )
            nc.vector.tensor_copy(
                vt_c[c][:, s, :, 0:HS],
                ps.rearrange("p (h d) -> p h d", h=HPC),
            )
            nc.gpsimd.memset(vt_c[c][:, s, :, HS:HS + 1], 1.0)

    # =====================================================================
    # Phase 2: attention + proj partials + per-chunk pair ReduceScatter
    # =====================================================================
    def finish_pair(av_sb, dr, attT):
        """Broadcast the reciprocal denominators across each head's 64
        partitions with a K=1 ones-matmul and normalize into attT (bf16)."""
        for hh in range(2):
            rb = ps_mm.tile([HS, TC], f32, name="rb", tag="mm")
            nc.tensor.matmul(
                rb, mc(ones_row), mc(dr[:, hh, :]), start=True, stop=True
            )
            nc.vector.tensor_mul(
                attT[hh * HS:(hh + 1) * HS, :], av_sb[0:HS, hh, :], rb
            )

    def attention_chunk(c):
        nkt = (c + 1) * NSUB
        attTs = []
        pending = None
        for pr in range(NDT):  # head pair = d-tile
            av_ps = [ps_av.tile([HS + 1, TC], f32, name="avp") for _ in range(2)]
            def av_mms(pi, ppt, p0, last):
                # columns below p0 get no contribution from this t_k tile
                # (fully above the diagonal); PSUM accumulation is
                # per-element so the shorter matmul leaves them untouched
                for hh in range(2):
                    nc.tensor.matmul(
                        av_ps[hh][:, p0:TC],
                        mc(vt_c[pi // NSUB][:, pi % NSUB, pr * 2 + hh, :]),
                        mc(ppt[:, hh, p0:TC]),
                        start=(pi == 0), stop=last,
                    )

            avq = []  # stagger AV matmuls 2 units behind exp+mask
            for i in range(nkt):
                m = i - c * NSUB
                # p0: first t_q column this t_k tile can attend to
                p0 = m * TS if m > 0 else 0
                sc2 = ps_sc.tile([TS, 2, TC], f32, name="sc2")
                for hh in range(2):
                    h0 = hh * HS
                    nc.tensor.matmul(
                        sc2[:, hh, p0:TC],
                        mc(kT_c[i // NSUB][h0:h0 + HS, pr,
                                           (i % NSUB) * TS:(i % NSUB + 1) * TS]),
                        mc(qT_c[c][h0:h0 + HS, pr, p0:TC]),
                        start=True, stop=True,
                    )
                pt2 = pt_pool.tile([TS, 2, TC], mdt, name="pt2")
                nc.scalar.activation(
                    out=pt2[:, :, p0:TC], in_=sc2[:, :, p0:TC],
                    func=AF.Exp, scale=SCALE,
                )
                if m >= 0:
                    # diagonal TS block: zero t_k > t_q within it
                    for hh in range(2):
                        nc.vector.tensor_mul(
                            pt2[:, hh, p0:p0 + TS], pt2[:, hh, p0:p0 + TS],
                            mask_sb[:, m, p0:p0 + TS],
                        )
                avq.append((i, pt2, p0))
                if len(avq) > 2:
                    av_mms(*avq.pop(0), last=False)
                if i == 2 and pending is not None:
                    finish_pair(*pending)
                    pending = None
            while avq:
                av_mms(*avq.pop(0), last=(len(avq) == 0))
            av_sb = avs_pool.tile([HS + 1, 2, TC], mdt, name="av_sb")
            for hh in range(2):
                nc.vector.tensor_copy(av_sb[:, hh, :], av_ps[hh])
            # 1/den on ScalarE as exp(-ln(den)): a [1, N] DVE reciprocal
            # would serialize ~8 cycles/element on a single lane
            dr = dr_pool.tile([1, 2, TC], mdt, name="dr")
            lden = dr_pool.tile([1, 2, TC], f32, name="lden", tag="lden")
            nc.scalar.activation(
                out=lden, in_=av_sb[HS:HS + 1, :, :], func=AF.Ln
            )
            nc.scalar.activation(out=dr, in_=lden, func=AF.Exp, scale=-1.0)
            attT = attT_pool.tile([128, TC], mdt, name="attT")
            attTs.append(attT)
            if pending is not None:
                finish_pair(*pending)
            pending = (av_sb, dr, attT)
        finish_pair(*pending)
        return attTs

    def proj_chunk(c, attTs):
        for s in range(NSUB):
            part = stage_pool.tile([128, E], mdt, name="part", tag="stg")
            for n in range(2):
                ps = ps_mm.tile([128, TC], f32, name="ps_pr", tag="mm")
                for dd in range(NDT):
                    nc.tensor.matmul(
                        ps, mc(attTs[dd][:, s * TS:(s + 1) * TS]),
                        mc(wp_sb[:, dd, n * TC:(n + 1) * TC]),
                        start=(dd == 0), stop=(dd == NDT - 1),
                    )
                nc.vector.tensor_copy(part[:, n * TC:(n + 1) * TC], ps)
            nc.sync.dma_start(out=ar_c[c][s * TS:(s + 1) * TS, :], in_=part)
        nc.gpsimd.collective_compute(
            "ReduceScatter", mybir.AluOpType.add, replica_groups=PAIRS,
            ins=[ar_c[c][:]], outs=[rs_c[c]],
        )

    # =====================================================================
    # Phase 3: residual + LN2 + FFN on this core's 256-token shard of chunk c
    # =====================================================================
    def ffn_chunk(c):
        x2_ts = []
        h2_ts = []
        for s in range(2):
            rs_sb = stage_pool.tile([128, E], mdt, name="rs_sb", tag="stg")
            nc.sync.dma_start(out=rs_sb, in_=rs_c[c][s * TS:(s + 1) * TS, :])
            x2_t = x2_pool.tile([128, E], f32, name="x2_t")
            nc.sync.dma_start(
                out=x2_t, in_=io["x_own"][c, s * TS:(s + 1) * TS, :]
            )
            nc.vector.tensor_add(x2_t, x2_t, rs_sb)
            x2_ts.append(x2_t)
            h2_t = h_pool.tile([128, E], mdt, name="h2_t", tag="h_t")
            layer_norm(x2_t, h2_t)
            h2_ts.append(h2_t)
        h2T = hT_pool.tile([ET, NET, TC // 2], mdt, name="h2T")
        transpose_cast(h2_ts, ln_sb["ln2g"], ln_sb["ln2b"], h2T, TC // 2)
        f1 = f1_pool.tile([FFN + 1, TC // 2], mdt, name="f1")
        nc.vector.memset(f1, 1.0)  # row FFN stays 1.0 (b2 matmul row)
        ps_f = ps_mm.tile([FFN, TC // 2], f32, name="ps_f", tag="mm")
        for k in range(NET):
            nc.tensor.matmul(
                ps_f, mc(w1_sb[:, k, :]), mc(h2T[:, k, :]),
                start=(k == 0), stop=(k == NET - 1),
            )
        nc.scalar.activation(
            out=f1[0:FFN, :], in_=ps_f, func=AF.Relu, bias=b1_sb, scale=1.0
        )
        for s in range(2):
            o_t = out_pool.tile([128, E], f32, name="o_t")
            for n in range(2):
                ps = ps_mm.tile([128, TC], f32, name="ps_o", tag="mm")
                nc.tensor.matmul(
                    ps, mc(f1[:, s * TS:(s + 1) * TS]),
                    mc(w2_sb[:, n * TC:(n + 1) * TC]),
                    start=True, stop=True,
                )
                nc.vector.tensor_add(
                    o_t[:, n * TC:(n + 1) * TC], ps,
                    x2_ts[s][:, n * TC:(n + 1) * TC],
                )
            nc.sync.dma_start(out=out[c, s * TS:(s + 1) * TS, :], in_=o_t)

    # ---- schedule: qkv all chunks; attention biggest-chunk-first so the
    # big ReduceScatters get the most overlap, with each chunk's FFN emitted
    # once a later RS is in flight so only the final small pieces are exposed.
    h0 = ln1_chunk(0)
    load_late_weights()  # after chunk 0's x loads in DMA-ordinal order
    qkv_chunk(0, h0)
    for c in range(1, NTC):
        qkv_chunk(c, ln1_chunk(c))
    order = list(range(NTC - 1, -1, -1))  # 3, 2, 1, 0
    for idx, c in enumerate(order):
        attTs = attention_chunk(c)
        proj_chunk(c, attTs)
        if idx >= 1:
            ffn_chunk(order[idx - 1])
    ffn_chunk(order[-1])


# =========================================================================
# Host side
# =========================================================================
def _make_masks(np_mdt):
    # masks[p, d, f] = 1 iff t_k <= t_q for the diagonal block at offset d,
    # i.e. f >= 128*d + p  (t_k = 128*i + p, t_q = 512*c + f, i = 4*c + d)
    m = np.zeros((TS, NSUB, TC), dtype=np.float32)
    for d in range(NSUB):
        for p in range(TS):
            m[p, d, d * TS + p:] = 1.0
    return m.astype(np_mdt)


_NC_CACHE = {}
RUN_KWARGS = {}      # test harness may set {"trace": True} for profiling
LAST_RESULT = None   # BassKernelResults of the most recent run


def kernel(x, wq, wk, wv, w_proj, b_proj, w1, b1, w2, b2, ln1_g, ln1_b, ln2_g,
           ln2_b):
    mode = MM_MODE
    np_mdt = _np_mdt(mode)
    if mode not in _NC_CACHE:
        _NC_CACHE[mode] = build(mode)
    nc = _NC_CACHE[mode]

    x = np.asarray(x, np.float32)
    bp = np.asarray(b_proj, np.float32)
    masks = _make_masks(np_mdt)
    identity = np.eye(TS, dtype=np.float32)
    w2e = np.concatenate([np.asarray(w2, np.float32),
                          np.asarray(b2, np.float32)[None, :]], axis=0)
    in_maps = []
    for core in range(NCORE):
        b, g = core // 2, core % 2
        sl = slice(g * DSL, (g + 1) * DSL)
        # rows this core owns after the per-chunk pair ReduceScatter
        x_own = np.stack(
            [x[b, c * TC + g * (TC // 2):c * TC + (g + 1) * (TC // 2), :]
             for c in range(NTC)]
        ) + bp[None, None, :]
        in_maps.append({
            "x": x[b],
            "x_own": x_own,
            "wq": np.asarray(wq, np.float32)[:, sl].astype(np_mdt),
            "wk": np.asarray(wk, np.float32)[:, sl].astype(np_mdt),
            "wv": np.asarray(wv, np.float32)[:, sl].astype(np_mdt),
            "wp": np.asarray(w_proj, np.float32)[sl, :].astype(np_mdt),
            "w1": np.asarray(w1, np.float32).astype(np_mdt),
            "w2e": w2e.astype(np_mdt),
            "b1": np.asarray(b1, np.float32)[:, None],
            "ln1g": np.asarray(ln1_g, np.float32)[:, None],
            "ln1b": np.asarray(ln1_b, np.float32)[:, None],
            "ln2g": np.asarray(ln2_g, np.float32)[:, None],
            "ln2b": np.asarray(ln2_b, np.float32)[:, None],
            "masks": masks,
            "ident": identity.astype(np_mdt),
        })
    global LAST_RESULT
    res = run_bass_kernel_spmd(nc, in_maps, list(range(NCORE)), **RUN_KWARGS)
    LAST_RESULT = res
    outp = np.empty((B, T, E), np.float32)
    for core in range(NCORE):
        b, g = core // 2, core % 2
        o = res.results[core]["out"]
        for c in range(NTC):
            r0 = c * TC + g * (TC // 2)
            outp[b, r0:r0 + TC // 2, :] = o[c]
    return outp
